# revision 17
# baseline (speedup 1.0000x reference)
"""Trainium2 Bass kernel for nn_AlternatingForecastModel.

2-layer LSTM (H=512) over S=2688 steps, B=512. Odd weeks feed the model's
previous prediction back as input feature 0. Data-parallel over batch:
8 cores x 64 rows, weights replicated, scan local per core.

Per core/step (bl=64): gates[bl, 2048] accumulate in PSUM via bf16 matmuls
with lhsT = transposed activations (curT [35,64] / hT chunks [128,64]) and
rhs = pre-transposed weights streamed at N=512. Layer-0 gates live in PSUM
partitions 0-63 (banks 0-3), layer-1 gates in partitions 64-127 (banks 4-7):
matmuls of the two layers target disjoint PE column groups and execute
concurrently (col tiling), with emission interleaved to pair them. Gate rows
are host-reordered to [i, f, o, g] so one sigmoid covers cols 0:1536.
Biases enter as hi+lo bf16 ones-rows (exact to ~2^-17). Elementwise (fp32)
on ACT/DVE; h_new transposed back via PE transposes into reused PSUM banks,
evacuated as bf16; pred = wout . h1 via M=1 matmuls giving predT [1, 64].
Emission software-pipelines: gates1's h1-part pairs with gates0, the next
step's Whh0-part pairs with gates1's h0-part.
"""

import numpy as np
import ml_dtypes

import concourse.bacc as bacc
import concourse.mybir as mybir
import concourse.tile as tile
from concourse.bass import ds
from concourse.bass_utils import run_bass_kernel_spmd

FP32 = mybir.dt.float32
BF16 = mybir.dt.bfloat16
AF = mybir.ActivationFunctionType

B, S, F = 512, 2688, 32
H = 512
G = 4 * H
WEEK = 672
NCORES = 8
BL = B // NCORES          # 64 batch rows per core
KX = F + 3                # 35: [feat0, x1..x31, flag, ones_hi, ones_lo]
U = 8                     # steps per sub-block (one x DMA)
STEPS_PER_IT = 2 * U      # 16
IT_PER_WEEK = WEEK // STEPS_PER_IT  # 42
NBLK = S // U             # 336
NIT = S // STEPS_PER_IT   # 168

_LAST_RESULTS = None


def _interleave(la, lb):
    """Alternate emission of two thunk lists (A/B PE col groups)."""
    n = max(len(la), len(lb))
    for i in range(n):
        if i < len(lb):
            lb[i]()
        if i < len(la):
            la[i]()


def _build(bout_val: float, trace: bool = False):
    nc = bacc.Bacc("TRN2")

    xaug_d = nc.declare_dram_parameter("xaug", [NBLK * KX, U * BL], BF16, isOutput=False)
    whh0t_d = nc.declare_dram_parameter("whh0t", [128, 4 * G], BF16, isOutput=False)
    wih1t_d = nc.declare_dram_parameter("wih1t", [128, 4 * G], BF16, isOutput=False)
    whh1t_d = nc.declare_dram_parameter("whh1t", [128, 4 * G], BF16, isOutput=False)
    wih0t_d = nc.declare_dram_parameter("wih0t", [128, G], BF16, isOutput=False)
    bias1_d = nc.declare_dram_parameter("bias1", [128, G], BF16, isOutput=False)
    woutt_d = nc.declare_dram_parameter("woutt", [128, 4], BF16, isOutput=False)
    ones_d = nc.declare_dram_parameter("ones", [128, BL], BF16, isOutput=False)
    zeros_d = nc.declare_dram_parameter("zeros", [128, 4 * BL], BF16, isOutput=False)
    identc_d = nc.declare_dram_parameter("identc", [128, BL], BF16, isOutput=False)
    out_d = nc.declare_dram_parameter("out", [NIT, STEPS_PER_IT * BL], FP32, isOutput=True)

    # SBUF
    whh0t = nc.alloc_sbuf_tensor("whh0t_s", [128, 4 * G], BF16)
    wih1t = nc.alloc_sbuf_tensor("wih1t_s", [128, 4 * G], BF16)
    whh1t = nc.alloc_sbuf_tensor("whh1t_s", [128, 4 * G], BF16)
    wih0t = nc.alloc_sbuf_tensor("wih0t_s", [128, G], BF16)
    bias1 = nc.alloc_sbuf_tensor("bias1_s", [128, G], BF16)
    woutt = nc.alloc_sbuf_tensor("woutt_s", [128, 4], BF16)
    ones = nc.alloc_sbuf_tensor("ones_s", [128, BL], BF16)
    identc = nc.alloc_sbuf_tensor("identc_s", [128, BL], BF16)

    xbuf = [nc.alloc_sbuf_tensor(f"xbuf{a}", [128, U * BL], BF16) for a in (0, 1)]
    h0T = [nc.alloc_sbuf_tensor(f"h0T{p}", [128, 4 * BL], BF16) for p in (0, 1)]
    h1T = [nc.alloc_sbuf_tensor(f"h1T{p}", [128, 4 * BL], BF16) for p in (0, 1)]
    # layer-0 elementwise state at partitions 0-63, layer-1 at 64-127
    c0 = nc.alloc_sbuf_tensor("c0", [BL, H], FP32)
    c1f = nc.alloc_sbuf_tensor("c1f", [128, H], FP32)
    sig0 = [nc.alloc_sbuf_tensor(f"sig0{p}", [BL, 3 * H], FP32) for p in (0, 1)]
    sig1 = [nc.alloc_sbuf_tensor(f"sig1{p}", [128, 3 * H], FP32) for p in (0, 1)]
    tg0 = [nc.alloc_sbuf_tensor(f"tg0{p}", [BL, H], FP32) for p in (0, 1)]
    tg1 = [nc.alloc_sbuf_tensor(f"tg1{p}", [128, H], FP32) for p in (0, 1)]
    tc0 = [nc.alloc_sbuf_tensor(f"tc0{p}", [BL, H], FP32) for p in (0, 1)]
    tc1 = [nc.alloc_sbuf_tensor(f"tc1{p}", [128, H], FP32) for p in (0, 1)]
    ta0 = [nc.alloc_sbuf_tensor(f"ta0{p}", [BL, H], FP32) for p in (0, 1)]
    tb0 = [nc.alloc_sbuf_tensor(f"tb0{p}", [BL, H], FP32) for p in (0, 1)]
    ta1 = [nc.alloc_sbuf_tensor(f"ta1{p}", [128, H], FP32) for p in (0, 1)]
    tb1 = [nc.alloc_sbuf_tensor(f"tb1{p}", [128, H], FP32) for p in (0, 1)]
    h0 = [nc.alloc_sbuf_tensor(f"h0{p}", [BL, H], BF16) for p in (0, 1)]
    h1 = [nc.alloc_sbuf_tensor(f"h1{p}", [128, H], BF16) for p in (0, 1)]
    outst = nc.alloc_sbuf_tensor("outst", [1, STEPS_PER_IT * BL], FP32)
    predl = nc.alloc_sbuf_tensor("predl", [1, BL], BF16)

    g0 = nc.alloc_psum_tensor("g0", [128, G], FP32)
    g1 = nc.alloc_psum_tensor("g1", [128, G], FP32)

    NS = G // 512  # 4 N-slices per gate vector

    # ---- thunk builders for PE matmul groups (A = layer0/parts 0-63,
    #      B = layer1/parts 64-127) ----

    def mm_whh0(u, first):
        """Whh0-part of gates0(t) emitted during step t-1. `early` (bank 1,
        chunks 0-1) may interleave mid-step; `late` (incl. all bank-0 MMs)
        must follow transpose0-half1 + CASTs, since bank 0 bytes 0-511 are
        the transpose scratch. start=True on the first write of each bank."""
        prev = (u + 1) % 2

        def f(k, ns):
            lhs = h0T[prev].ap()[:, k * BL:(k + 1) * BL]
            def g(k=k, ns=ns, lhs=lhs):
                nc.tensor.matmul(
                    g0.ap()[0:BL, ns * 512:(ns + 1) * 512],
                    lhs,
                    whh0t.ap()[:, k * G + ns * 512:k * G + (ns + 1) * 512],
                    start=(first and k == 0), stop=False)
            return g
        early = [f(0, 1), f(1, 1)]
        late = ([f(k, 0) for k in range(4)]
                + [f(k, ns) for ns in (2, 3) for k in (0, 1)]
                + [f(k, ns) for ns in (1, 2, 3) for k in (2, 3)])
        return early, late

    def mm_xside(u, xb):
        slot = (u % U) * BL
        xl = xb.ap()[0:128, slot:slot + BL]
        out = []
        for ns in range(NS):
            def f(ns=ns):
                nc.tensor.matmul(g0.ap()[0:BL, ns * 512:(ns + 1) * 512],
                                 xl, wih0t.ap()[:, ns * 512:(ns + 1) * 512],
                                 start=False, stop=(ns == NS - 1))
            out.append(f)
        return out

    def mm_bias1(u):
        out = []
        for ns in range(NS):
            def f(ns=ns):
                nc.tensor.matmul(g1.ap()[64:128, ns * 512:(ns + 1) * 512],
                                 ones.ap()[0:128, :],
                                 bias1.ap()[:, ns * 512:(ns + 1) * 512],
                                 start=True, stop=False)
            out.append(f)
        return out

    def mm_whh1(u):
        prev = (u + 1) % 2
        out = []
        for k in range(4):
            lhs = h1T[prev].ap()[:, k * BL:(k + 1) * BL]
            for ns in range(NS):
                def f(k=k, ns=ns, lhs=lhs):
                    nc.tensor.matmul(
                        g1.ap()[64:128, ns * 512:(ns + 1) * 512],
                        lhs,
                        whh1t.ap()[:, k * G + ns * 512:k * G + (ns + 1) * 512],
                        start=False, stop=False)
                out.append(f)
        return out

    def mm_wih1(u):
        """Wih1-part of gates1(t). k01: ns-major over chunks 0-1 (gated on
        CAST-h0a); k23a: ns0-1 of chunks 2-3 (sig1-h0 gate); k23b: rest."""
        par = u % 2

        def f(k, ns):
            lhs = h0T[par].ap()[:, k * BL:(k + 1) * BL]
            def g(k=k, ns=ns, lhs=lhs):
                nc.tensor.matmul(
                    g1.ap()[64:128, ns * 512:(ns + 1) * 512],
                    lhs,
                    wih1t.ap()[:, k * G + ns * 512:k * G + (ns + 1) * 512],
                    start=False, stop=(k == 3 and ns == 3))
            return g
        k01 = [f(k, ns) for ns in range(NS) for k in (0, 1)]
        k23a = [f(k, ns) for ns in (0, 1) for k in (2, 3)]
        k23b = [f(k, ns) for ns in (2, 3) for k in (2, 3)]
        return k01, k23a, k23b

    # ---- elementwise ----

    HB = 256  # hidden half-block; gate rows are [i,f,o,g]x2 halves of 1024

    g0b = g0.bitcast(BF16)   # [128, 4096] bf16 view for transpose targets
    g1b = g1.bitcast(BF16)

    def emit_ew0_half(par, hb):
        """Layer-0 elementwise for one hidden half."""
        sg = sig0[par].ap()
        if True:
            gofs = hb * 1024
            nc.scalar.activation(sg[:, hb * 768:hb * 768 + HB],
                                 g0.ap()[0:BL, gofs:gofs + HB], AF.Sigmoid)
            nc.scalar.activation(tg0[par].ap()[:, hb * HB:hb * HB + HB],
                                 g0.ap()[0:BL, gofs + 768:gofs + 1024], AF.Tanh)
            nc.scalar.activation(sg[:, hb * 768 + HB:hb * 768 + 768],
                                 g0.ap()[0:BL, gofs + HB:gofs + 768], AF.Sigmoid)
            nc.vector.tensor_mul(ta0[par].ap()[:, hb * HB:hb * HB + HB],
                                 sg[:, hb * 768:hb * 768 + HB],
                                 tg0[par].ap()[:, hb * HB:hb * HB + HB])
            nc.vector.tensor_mul(tb0[par].ap()[:, hb * HB:hb * HB + HB],
                                 sg[:, hb * 768 + HB:hb * 768 + 2 * HB],
                                 c0.ap()[:, hb * HB:hb * HB + HB])
            nc.vector.tensor_add(c0.ap()[:, hb * HB:hb * HB + HB],
                                 ta0[par].ap()[:, hb * HB:hb * HB + HB],
                                 tb0[par].ap()[:, hb * HB:hb * HB + HB])
            nc.scalar.activation(tc0[par].ap()[:, hb * HB:hb * HB + HB],
                                 c0.ap()[:, hb * HB:hb * HB + HB], AF.Tanh)
            nc.vector.tensor_mul(h0[par].ap()[:, hb * HB:hb * HB + HB],
                                 sg[:, hb * 768 + 2 * HB:hb * 768 + 3 * HB],
                                 tc0[par].ap()[:, hb * HB:hb * HB + HB])

    def emit_transpose0_half(par, hb):
        for k in (2 * hb, 2 * hb + 1):
            nc.tensor.transpose(g0b.ap()[0:128, k * BL:(k + 1) * BL],
                                h0[par].ap()[0:BL, k * 128:(k + 1) * 128],
                                identc.ap()[0:BL, :])
        nc.vector.tensor_copy(
            h0T[par].ap()[:, hb * 2 * BL:(hb + 1) * 2 * BL],
            g0b.ap()[0:128, hb * 2 * BL:(hb + 1) * 2 * BL])

    def emit_ew1_half(par, hb):
        sg = sig1[par].ap()[64:128, :]
        tg = tg1[par].ap()[64:128, :]
        cc = c1f.ap()[64:128, :]
        aa = ta1[par].ap()[64:128, :]
        bb = tb1[par].ap()[64:128, :]
        tcc = tc1[par].ap()[64:128, :]
        hh = h1[par].ap()[64:128, :]
        if True:
            gofs = hb * 1024
            nc.scalar.activation(sg[:, hb * 768:hb * 768 + HB],
                                 g1.ap()[64:128, gofs:gofs + HB], AF.Sigmoid)
            nc.scalar.activation(tg[:, hb * HB:hb * HB + HB],
                                 g1.ap()[64:128, gofs + 768:gofs + 1024],
                                 AF.Tanh)
            nc.scalar.activation(sg[:, hb * 768 + HB:hb * 768 + 768],
                                 g1.ap()[64:128, gofs + HB:gofs + 768],
                                 AF.Sigmoid)
            nc.vector.tensor_mul(aa[:, hb * HB:hb * HB + HB],
                                 sg[:, hb * 768:hb * 768 + HB],
                                 tg[:, hb * HB:hb * HB + HB])
            nc.vector.tensor_mul(bb[:, hb * HB:hb * HB + HB],
                                 sg[:, hb * 768 + HB:hb * 768 + 2 * HB],
                                 cc[:, hb * HB:hb * HB + HB])
            nc.vector.tensor_add(cc[:, hb * HB:hb * HB + HB],
                                 aa[:, hb * HB:hb * HB + HB],
                                 bb[:, hb * HB:hb * HB + HB])
            nc.scalar.activation(tcc[:, hb * HB:hb * HB + HB],
                                 cc[:, hb * HB:hb * HB + HB], AF.Tanh)
            nc.vector.tensor_mul(hh[:, hb * HB:hb * HB + HB],
                                 sg[:, hb * 768 + 2 * HB:hb * 768 + 3 * HB],
                                 tcc[:, hb * HB:hb * HB + HB])

    def emit_transpose1_half(par, hb):
        for k in (2 * hb, 2 * hb + 1):
            nc.tensor.transpose(g1b.ap()[0:128, k * BL:(k + 1) * BL],
                                h1[par].ap()[64:128, k * 128:(k + 1) * 128],
                                identc.ap()[64:128, :])
        nc.vector.tensor_copy(
            h1T[par].ap()[:, hb * 2 * BL:(hb + 1) * 2 * BL],
            g1b.ap()[0:128, hb * 2 * BL:(hb + 1) * 2 * BL])

    def emit_pred_mms(u, ks):
        par = u % 2
        pps = g1.ap()[0:1, 512:512 + BL]
        for k in ks:
            nc.tensor.matmul(pps, woutt.ap()[:, k:k + 1],
                             h1T[par].ap()[:, k * BL:(k + 1) * BL],
                             start=(k == 0), stop=(k == 3))

    def emit_pred_writes(u, pred_week, pred_dst):
        pps = g1.ap()[0:1, 512:512 + BL]
        nc.vector.tensor_scalar_add(outst.ap()[0:1, u * BL:(u + 1) * BL], pps,
                                    bout_val)
        if pred_week:
            dst_t, dst_col = pred_dst
            nc.vector.tensor_scalar_add(dst_t.ap()[0:1, dst_col:dst_col + BL],
                                        pps, bout_val)
        else:
            nc.vector.tensor_scalar_add(predl.ap(), pps, bout_val)

    def pred_dst_for(u, pred_week):
        if not pred_week:
            return None
        if (u % U) < U - 1:
            return (xbuf[u // U], ((u % U) + 1) * BL)
        if u < STEPS_PER_IT - 1:
            return (xbuf[1 - u // U], 0)
        return (xbuf[0], 0)

    with tile.TileContext(nc) as tc:
        # ---- preamble: weights, constants, state init ----
        nc.gpsimd.memset(c0.ap(), 0.0)
        nc.gpsimd.memset(c1f.ap(), 0.0)
        nc.sync.dma_start(out=whh0t.ap(), in_=whh0t_d.ap())
        nc.sync.dma_start(out=wih1t.ap(), in_=wih1t_d.ap())
        nc.sync.dma_start(out=whh1t.ap(), in_=whh1t_d.ap())
        nc.sync.dma_start(out=wih0t.ap(), in_=wih0t_d.ap())
        nc.sync.dma_start(out=bias1.ap(), in_=bias1_d.ap())
        nc.sync.dma_start(out=woutt.ap(), in_=woutt_d.ap())
        nc.sync.dma_start(out=ones.ap(), in_=ones_d.ap())
        nc.sync.dma_start(out=identc.ap(), in_=identc_d.ap())
        for p in (0, 1):
            nc.sync.dma_start(out=h0T[p].ap(), in_=zeros_d.ap())
            nc.sync.dma_start(out=h1T[p].ap(), in_=zeros_d.ap())
            nc.sync.dma_start(out=xbuf[p].ap()[:, 0:4 * BL], in_=zeros_d.ap())
            nc.sync.dma_start(out=xbuf[p].ap()[:, 4 * BL:8 * BL],
                              in_=zeros_d.ap())

        def week_loop(week, pred_week):
            blk_base = week * WEEK // U
            it_base = week * WEEK // STEPS_PER_IT

            def body(i):
                for a in (0, 1):
                    lo = 1 if pred_week else 0
                    nc.sync.dma_start(
                        out=xbuf[a].ap()[lo:KX, :],
                        in_=xaug_d.ap()[ds((blk_base + 2 * i + a) * KX + lo,
                                           KX - lo), :])
                run = lambda ts: [f() for f in ts]

                def step_head(u):
                    """transposes of h1(u-1), pred(u-1), staging writes —
                    runs right after EW1(u-1) completes. In original-data
                    weeks the next x-side matmuls fill the wait for the
                    second h1 half."""
                    up, parp = u - 1, (u - 1) % 2
                    emit_transpose1_half(parp, 0)
                    emit_pred_mms(up, (0, 1))
                    emit_transpose1_half(parp, 1)
                    emit_pred_mms(up, (2, 3))
                    emit_pred_writes(up, pred_week,
                                     pred_dst_for(up, pred_week))

                n_early, n_late = mm_whh0(0, True)
                run(n_early + n_late)
                _interleave(mm_xside(0, xbuf[0]), mm_bias1(0))
                for u in range(STEPS_PER_IT):
                    par = u % 2
                    emit_ew0_half(par, 0)       # ACT/DVE
                    emit_ew0_half(par, 1)
                    run(mm_whh1(u))             # PE fills the EW0 window
                    emit_transpose0_half(par, 0)
                    w_k01, w_k23a, w_k23b = mm_wih1(u)
                    if u + 1 < STEPS_PER_IT:
                        n_early, n_late = mm_whh0(u + 1, True)
                    else:
                        n_early, n_late = [], []
                    _interleave(w_k01, n_early)
                    emit_transpose0_half(par, 1)
                    run(w_k23a)
                    emit_ew1_half(par, 0)       # ACT/DVE
                    run(w_k23b)                 # gates1-h1 producers
                    emit_ew1_half(par, 1)
                    run(n_late)                 # fills the EW1 window
                    if u + 1 < STEPS_PER_IT:
                        step_head(u + 1)
                        _interleave(mm_xside(u + 1, xbuf[(u + 1) // U]),
                                    mm_bias1(u + 1))
                step_head(STEPS_PER_IT)
                nc.sync.dma_start(out=out_d.ap()[ds(it_base + i, 1), :],
                                  in_=outst.ap())

            with tc.For_i(0, IT_PER_WEEK, 1,
                          hint_engines=(mybir.EngineType.PE,
                                        mybir.EngineType.Activation,
                                        mybir.EngineType.DVE)) as i:
                body(i)

        week_loop(0, False)
        # pred(671) -> feat0 slot for t=672
        nc.vector.tensor_copy(xbuf[0].ap()[0:1, 0:BL], predl.ap())
        week_loop(1, True)
        week_loop(2, False)
        nc.vector.tensor_copy(xbuf[0].ap()[0:1, 0:BL], predl.ap())
        week_loop(3, True)

    nc.compile()
    return nc


def _prep_inputs(x, Wih0, Whh0, bih0, bhh0, Wih1, Whh1, bih1, bhh1, Wout, bout):
    """Host-side reshapes: gate reorder to [i,f,o,g], weight transposes,
    hi/lo bias split, per-core xaug staging layout. bf16 matmul operands."""
    f32 = np.float32
    bf16 = ml_dtypes.bfloat16
    HB = 256
    blocks = []
    for hb in range(2):
        blocks += [np.arange(0, 512)[hb*HB:(hb+1)*HB],          # i half
                   np.arange(512, 1024)[hb*HB:(hb+1)*HB],       # f half
                   np.arange(1536, 2048)[hb*HB:(hb+1)*HB],      # o half
                   np.arange(1024, 1536)[hb*HB:(hb+1)*HB]]      # g half
    perm = np.concatenate(blocks)

    def wT(w):  # [G, 512] -> [128, 4*G] chunk-k at cols [G*k, G*k+G)
        t = np.ascontiguousarray(w[perm].T.astype(f32))          # [512, G]
        return np.ascontiguousarray(
            t.reshape(4, 128, G).transpose(1, 0, 2).reshape(128, 4 * G)
        ).astype(bf16)

    def hilo(v):  # [G] fp32 -> [2, G] bf16 rows summing to ~v
        hi = v.astype(bf16).astype(f32)
        lo = (v - hi).astype(bf16)
        return np.stack([hi.astype(bf16), lo], axis=0)

    whh0t = wT(Whh0)
    wih1t = wT(Wih1)
    whh1t = wT(Whh1)
    bias0 = hilo((bih0 + bhh0)[perm].astype(f32))                # [2, G] bf16
    bias1 = np.zeros((128, G), bf16)
    bias1[0:2] = hilo((bih1 + bhh1)[perm].astype(f32))
    wih0p = Wih0[perm].astype(f32)                               # [G, 33]
    wih0t = np.zeros((128, G), bf16)
    wih0t[0:F + 1] = wih0p.T.astype(bf16)
    wih0t[F + 1:F + 3] = bias0
    woutt = np.ascontiguousarray(Wout.reshape(4, 128).T.astype(f32)).astype(bf16)

    onesp = np.zeros((128, BL), bf16)
    onesp[0:2] = 1.0

    tw = np.arange(S) // WEEK
    mask = np.where((tw % 2 == 0) & ((tw + 1) * WEEK <= S), 0.0, 1.0)
    flag = np.where((mask == 0.0) | (np.arange(S) == 0), 0.0, 1.0).astype(f32)

    xaugs = []
    for c in range(NCORES):
        xc = x[c * BL:(c + 1) * BL].astype(f32)        # [BL, S, F]
        arr = np.empty((S, KX, BL), f32)
        arr[:, 0, :] = xc[:, :, 0].T
        arr[:, 1:F, :] = xc[:, :, 1:].transpose(1, 2, 0)
        arr[:, F, :] = flag[:, None]
        arr[:, F + 1, :] = 1.0
        arr[:, F + 2, :] = 1.0
        a = arr.reshape(NBLK, U, KX, BL).transpose(0, 2, 1, 3)
        xaugs.append(np.ascontiguousarray(
            a.reshape(NBLK * KX, U * BL)).astype(bf16))

    shared = {
        "whh0t": whh0t, "wih1t": wih1t, "whh1t": whh1t, "wih0t": wih0t,
        "bias1": np.ascontiguousarray(bias1), "woutt": woutt,
        "ones": onesp, "zeros": np.zeros((128, 4 * BL), bf16),
        "identc": np.ascontiguousarray(
            np.tile(np.eye(BL, dtype=f32), (2, 1))).astype(bf16),
    }
    in_maps = [dict(shared, xaug=xaugs[c]) for c in range(NCORES)]
    return in_maps, float(np.asarray(bout).reshape(-1)[0])


def kernel(x, Wih0, Whh0, bih0, bhh0, Wih1, Whh1, bih1, bhh1, Wout, bout,
           _trace=False):
    global _LAST_RESULTS
    x = np.asarray(x)
    in_maps, bout_val = _prep_inputs(
        x, np.asarray(Wih0), np.asarray(Whh0), np.asarray(bih0),
        np.asarray(bhh0), np.asarray(Wih1), np.asarray(Whh1),
        np.asarray(bih1), np.asarray(bhh1), np.asarray(Wout),
        np.asarray(bout))
    nc = _build(bout_val, trace=_trace)
    res = run_bass_kernel_spmd(nc, in_maps, core_ids=list(range(NCORES)),
                               trace=_trace)
    _LAST_RESULTS = res
    out = np.empty((B, S, 1), np.float32)
    for c in range(NCORES):
        oc = res.results[c]["out"].reshape(S, BL)     # [S, BL]
        out[c * BL:(c + 1) * BL, :, 0] = oc.T
    return out


# revision 18
# speedup vs baseline: 1.2198x; 1.2198x over previous
"""Trainium2 Bass kernel for nn_AlternatingForecastModel.

2-layer LSTM (H=512) over S=2688 steps, B=512. Odd weeks feed the model's
previous prediction back as input feature 0. Data-parallel over batch:
8 cores x 64 rows, weights replicated, scan local per core.

Per core/step (bl=64): gates[bl, 2048] accumulate in PSUM via bf16 matmuls
with lhsT = transposed activations (curT [35,64] / hT chunks [128,64]) and
rhs = pre-transposed weights streamed at N=512. Layer-0 gates live in PSUM
partitions 0-63 (banks 0-3), layer-1 gates in partitions 64-127 (banks 4-7):
matmuls of the two layers target disjoint PE column groups and execute
concurrently (col tiling), with emission interleaved to pair them. Gate rows
are host-reordered to [i, f, o, g] so one sigmoid covers cols 0:1536.
Biases enter as hi+lo bf16 ones-rows (exact to ~2^-17). Elementwise (fp32)
on ACT/DVE; h_new transposed back via PE transposes into reused PSUM banks,
evacuated as bf16; pred = wout . h1 via M=1 matmuls giving predT [1, 64].
Emission software-pipelines: gates1's h1-part pairs with gates0, the next
step's Whh0-part pairs with gates1's h0-part.
"""

import numpy as np
import ml_dtypes

import concourse.bacc as bacc
import concourse.mybir as mybir
import concourse.tile as tile
from concourse.bass import ds
from concourse.bass_utils import run_bass_kernel_spmd

FP32 = mybir.dt.float32
BF16 = mybir.dt.bfloat16
AF = mybir.ActivationFunctionType

B, S, F = 512, 2688, 32
H = 512
G = 4 * H
WEEK = 672
NCORES = 8
BL = B // NCORES          # 64 batch rows per core
KX = F + 3                # 35: [feat0, x1..x31, flag, ones_hi, ones_lo]
U = 8                     # steps per sub-block (one x DMA)
STEPS_PER_IT = 2 * U      # 16
IT_PER_WEEK = WEEK // STEPS_PER_IT  # 42
NBLK = S // U             # 336
NIT = S // STEPS_PER_IT   # 168

_LAST_RESULTS = None


def _interleave(la, lb):
    """Alternate emission of two thunk lists (A/B PE col groups)."""
    n = max(len(la), len(lb))
    for i in range(n):
        if i < len(lb):
            lb[i]()
        if i < len(la):
            la[i]()


def _build(bout_val: float, trace: bool = False):
    nc = bacc.Bacc("TRN2")

    xaug_d = nc.declare_dram_parameter("xaug", [NBLK * KX, U * BL], BF16, isOutput=False)
    whh0t_d = nc.declare_dram_parameter("whh0t", [128, 4 * G], BF16, isOutput=False)
    wih1t_d = nc.declare_dram_parameter("wih1t", [128, 4 * G], BF16, isOutput=False)
    whh1t_d = nc.declare_dram_parameter("whh1t", [128, 4 * G], BF16, isOutput=False)
    wih0t_d = nc.declare_dram_parameter("wih0t", [128, G], BF16, isOutput=False)
    bias1_d = nc.declare_dram_parameter("bias1", [128, G], BF16, isOutput=False)
    woutt_d = nc.declare_dram_parameter("woutt", [128, 4], BF16, isOutput=False)
    ones_d = nc.declare_dram_parameter("ones", [128, BL], BF16, isOutput=False)
    zeros_d = nc.declare_dram_parameter("zeros", [128, 4 * BL], BF16, isOutput=False)
    identc_d = nc.declare_dram_parameter("identc", [128, BL], BF16, isOutput=False)
    out_d = nc.declare_dram_parameter("out", [NIT, STEPS_PER_IT * BL], FP32, isOutput=True)

    # SBUF
    whh0t = nc.alloc_sbuf_tensor("whh0t_s", [128, 4 * G], BF16)
    wih1t = nc.alloc_sbuf_tensor("wih1t_s", [128, 4 * G], BF16)
    whh1t = nc.alloc_sbuf_tensor("whh1t_s", [128, 4 * G], BF16)
    wih0t = nc.alloc_sbuf_tensor("wih0t_s", [128, G], BF16)
    bias1 = nc.alloc_sbuf_tensor("bias1_s", [128, G], BF16)
    woutt = nc.alloc_sbuf_tensor("woutt_s", [128, 4], BF16)
    ones = nc.alloc_sbuf_tensor("ones_s", [128, BL], BF16)
    identc = nc.alloc_sbuf_tensor("identc_s", [128, BL], BF16)

    xbuf = [nc.alloc_sbuf_tensor(f"xbuf{a}", [128, U * BL], BF16) for a in (0, 1)]
    h0T = [nc.alloc_sbuf_tensor(f"h0T{p}", [128, 4 * BL], BF16) for p in (0, 1)]
    h1T = [nc.alloc_sbuf_tensor(f"h1T{p}", [128, 4 * BL], BF16) for p in (0, 1)]
    # layer-0 elementwise state at partitions 0-63, layer-1 at 64-127
    c0 = nc.alloc_sbuf_tensor("c0", [BL, H], FP32)
    c1f = nc.alloc_sbuf_tensor("c1f", [128, H], FP32)
    sig0 = [nc.alloc_sbuf_tensor(f"sig0{p}", [BL, 3 * H], FP32) for p in (0, 1)]
    sig1 = [nc.alloc_sbuf_tensor(f"sig1{p}", [128, 3 * H], FP32) for p in (0, 1)]
    tg0 = [nc.alloc_sbuf_tensor(f"tg0{p}", [BL, H], FP32) for p in (0, 1)]
    tg1 = [nc.alloc_sbuf_tensor(f"tg1{p}", [128, H], FP32) for p in (0, 1)]
    tc0 = [nc.alloc_sbuf_tensor(f"tc0{p}", [BL, H], FP32) for p in (0, 1)]
    tc1 = [nc.alloc_sbuf_tensor(f"tc1{p}", [128, H], FP32) for p in (0, 1)]
    ta0 = [nc.alloc_sbuf_tensor(f"ta0{p}", [BL, H], FP32) for p in (0, 1)]
    tb0 = [nc.alloc_sbuf_tensor(f"tb0{p}", [BL, H], FP32) for p in (0, 1)]
    ta1 = [nc.alloc_sbuf_tensor(f"ta1{p}", [128, H], FP32) for p in (0, 1)]
    tb1 = [nc.alloc_sbuf_tensor(f"tb1{p}", [128, H], FP32) for p in (0, 1)]
    h0 = [nc.alloc_sbuf_tensor(f"h0{p}", [BL, H], BF16) for p in (0, 1)]
    h1 = [nc.alloc_sbuf_tensor(f"h1{p}", [128, H], BF16) for p in (0, 1)]
    outst = nc.alloc_sbuf_tensor("outst", [1, STEPS_PER_IT * BL], FP32)
    predl = nc.alloc_sbuf_tensor("predl", [1, BL], BF16)

    g0 = nc.alloc_psum_tensor("g0", [128, G], FP32)
    g1 = nc.alloc_psum_tensor("g1", [128, G], FP32)

    NS = G // 512  # 4 N-slices per gate vector

    # ---- thunk builders for PE matmul groups (A = layer0/parts 0-63,
    #      B = layer1/parts 64-127) ----

    def mm_whh0(u, first):
        """Whh0-part of gates0(t) emitted during step t-1. `early` (bank 1,
        chunks 0-1) may interleave mid-step; `late` (incl. all bank-0 MMs)
        must follow transpose0-half1 + CASTs, since bank 0 bytes 0-511 are
        the transpose scratch. start=True on the first write of each bank."""
        prev = (u + 1) % 2

        def f(k, ns):
            lhs = h0T[prev].ap()[:, k * BL:(k + 1) * BL]
            def g(k=k, ns=ns, lhs=lhs):
                nc.tensor.matmul(
                    g0.ap()[0:BL, ns * 512:(ns + 1) * 512],
                    lhs,
                    whh0t.ap()[:, k * G + ns * 512:k * G + (ns + 1) * 512],
                    start=(first and k == 0), stop=False)
            return g
        early = [f(0, 1), f(1, 1)]
        late = ([f(k, 0) for k in range(4)]
                + [f(k, ns) for ns in (2, 3) for k in (0, 1)]
                + [f(k, ns) for ns in (1, 2, 3) for k in (2, 3)])
        return early, late

    def mm_xside(u, xb):
        slot = (u % U) * BL
        xl = xb.ap()[0:128, slot:slot + BL]
        out = []
        for ns in range(NS):
            def f(ns=ns):
                nc.tensor.matmul(g0.ap()[0:BL, ns * 512:(ns + 1) * 512],
                                 xl, wih0t.ap()[:, ns * 512:(ns + 1) * 512],
                                 start=False, stop=(ns == NS - 1))
            out.append(f)
        return out

    def mm_bias1(u):
        out = []
        for ns in range(NS):
            def f(ns=ns):
                nc.tensor.matmul(g1.ap()[64:128, ns * 512:(ns + 1) * 512],
                                 ones.ap()[0:128, :],
                                 bias1.ap()[:, ns * 512:(ns + 1) * 512],
                                 start=True, stop=False)
            out.append(f)
        return out

    def mm_whh1(u):
        prev = (u + 1) % 2
        out = []
        for k in range(4):
            lhs = h1T[prev].ap()[:, k * BL:(k + 1) * BL]
            for ns in range(NS):
                def f(k=k, ns=ns, lhs=lhs):
                    nc.tensor.matmul(
                        g1.ap()[64:128, ns * 512:(ns + 1) * 512],
                        lhs,
                        whh1t.ap()[:, k * G + ns * 512:k * G + (ns + 1) * 512],
                        start=False, stop=False)
                out.append(f)
        return out

    def mm_wih1(u):
        """Wih1-part of gates1(t). k01: ns-major over chunks 0-1 (gated on
        CAST-h0a); k23a: ns0-1 of chunks 2-3 (sig1-h0 gate); k23b: rest."""
        par = u % 2

        def f(k, ns):
            lhs = h0T[par].ap()[:, k * BL:(k + 1) * BL]
            def g(k=k, ns=ns, lhs=lhs):
                nc.tensor.matmul(
                    g1.ap()[64:128, ns * 512:(ns + 1) * 512],
                    lhs,
                    wih1t.ap()[:, k * G + ns * 512:k * G + (ns + 1) * 512],
                    start=False, stop=(k == 3 and ns == 3))
            return g
        k01 = [f(k, ns) for ns in range(NS) for k in (0, 1)]
        k23a = [f(k, ns) for ns in (0, 1) for k in (2, 3)]
        k23b = [f(k, ns) for ns in (2, 3) for k in (2, 3)]
        return k01, k23a, k23b

    # ---- elementwise ----

    HB = 256  # hidden half-block; gate rows are [i,f,o,g]x2 halves of 1024

    g0b = g0.bitcast(BF16)   # [128, 4096] bf16 view for transpose targets
    g1b = g1.bitcast(BF16)

    def emit_ew0_half(par, hb):
        """Layer-0 elementwise for one hidden half."""
        sg = sig0[par].ap()
        if True:
            gofs = hb * 1024
            nc.scalar.activation(sg[:, hb * 768:hb * 768 + 768],
                                 g0.ap()[0:BL, gofs:gofs + 768], AF.Sigmoid)
            nc.scalar.activation(tg0[par].ap()[:, hb * HB:hb * HB + HB],
                                 g0.ap()[0:BL, gofs + 768:gofs + 1024], AF.Tanh)
            nc.vector.tensor_mul(ta0[par].ap()[:, hb * HB:hb * HB + HB],
                                 sg[:, hb * 768:hb * 768 + HB],
                                 tg0[par].ap()[:, hb * HB:hb * HB + HB])
            nc.vector.tensor_mul(tb0[par].ap()[:, hb * HB:hb * HB + HB],
                                 sg[:, hb * 768 + HB:hb * 768 + 2 * HB],
                                 c0.ap()[:, hb * HB:hb * HB + HB])
            nc.vector.tensor_add(c0.ap()[:, hb * HB:hb * HB + HB],
                                 ta0[par].ap()[:, hb * HB:hb * HB + HB],
                                 tb0[par].ap()[:, hb * HB:hb * HB + HB])
            nc.scalar.activation(tc0[par].ap()[:, hb * HB:hb * HB + HB],
                                 c0.ap()[:, hb * HB:hb * HB + HB], AF.Tanh)
            nc.vector.tensor_mul(h0[par].ap()[:, hb * HB:hb * HB + HB],
                                 sg[:, hb * 768 + 2 * HB:hb * 768 + 3 * HB],
                                 tc0[par].ap()[:, hb * HB:hb * HB + HB])

    def emit_transpose0_half(par, hb):
        for k in (2 * hb, 2 * hb + 1):
            nc.tensor.transpose(g0b.ap()[0:128, k * BL:(k + 1) * BL],
                                h0[par].ap()[0:BL, k * 128:(k + 1) * 128],
                                identc.ap()[0:BL, :])
        nc.vector.tensor_copy(
            h0T[par].ap()[:, hb * 2 * BL:(hb + 1) * 2 * BL],
            g0b.ap()[0:128, hb * 2 * BL:(hb + 1) * 2 * BL])

    def emit_ew1_half(par, hb):
        sg = sig1[par].ap()[64:128, :]
        tg = tg1[par].ap()[64:128, :]
        cc = c1f.ap()[64:128, :]
        aa = ta1[par].ap()[64:128, :]
        bb = tb1[par].ap()[64:128, :]
        tcc = tc1[par].ap()[64:128, :]
        hh = h1[par].ap()[64:128, :]
        if True:
            gofs = hb * 1024
            nc.scalar.activation(sg[:, hb * 768:hb * 768 + 768],
                                 g1.ap()[64:128, gofs:gofs + 768], AF.Sigmoid)
            nc.scalar.activation(tg[:, hb * HB:hb * HB + HB],
                                 g1.ap()[64:128, gofs + 768:gofs + 1024],
                                 AF.Tanh)
            nc.vector.tensor_mul(aa[:, hb * HB:hb * HB + HB],
                                 sg[:, hb * 768:hb * 768 + HB],
                                 tg[:, hb * HB:hb * HB + HB])
            nc.vector.tensor_mul(bb[:, hb * HB:hb * HB + HB],
                                 sg[:, hb * 768 + HB:hb * 768 + 2 * HB],
                                 cc[:, hb * HB:hb * HB + HB])
            nc.vector.tensor_add(cc[:, hb * HB:hb * HB + HB],
                                 aa[:, hb * HB:hb * HB + HB],
                                 bb[:, hb * HB:hb * HB + HB])
            nc.scalar.activation(tcc[:, hb * HB:hb * HB + HB],
                                 cc[:, hb * HB:hb * HB + HB], AF.Tanh)
            nc.vector.tensor_mul(hh[:, hb * HB:hb * HB + HB],
                                 sg[:, hb * 768 + 2 * HB:hb * 768 + 3 * HB],
                                 tcc[:, hb * HB:hb * HB + HB])

    def emit_transpose1_half(par, hb):
        for k in (2 * hb, 2 * hb + 1):
            nc.tensor.transpose(g1b.ap()[0:128, k * BL:(k + 1) * BL],
                                h1[par].ap()[64:128, k * 128:(k + 1) * 128],
                                identc.ap()[64:128, :])
        nc.vector.tensor_copy(
            h1T[par].ap()[:, hb * 2 * BL:(hb + 1) * 2 * BL],
            g1b.ap()[0:128, hb * 2 * BL:(hb + 1) * 2 * BL])

    def emit_pred_mms(u, ks):
        par = u % 2
        pps = g1.ap()[0:1, 512:512 + BL]
        for k in ks:
            nc.tensor.matmul(pps, woutt.ap()[:, k:k + 1],
                             h1T[par].ap()[:, k * BL:(k + 1) * BL],
                             start=(k == 0), stop=(k == 3))

    def emit_pred_writes(u, pred_week, pred_dst):
        pps = g1.ap()[0:1, 512:512 + BL]
        nc.vector.tensor_scalar_add(outst.ap()[0:1, u * BL:(u + 1) * BL], pps,
                                    bout_val)
        if pred_week:
            dst_t, dst_col = pred_dst
            nc.vector.tensor_scalar_add(dst_t.ap()[0:1, dst_col:dst_col + BL],
                                        pps, bout_val)
        else:
            nc.vector.tensor_scalar_add(predl.ap(), pps, bout_val)

    def pred_dst_for(u, pred_week):
        if not pred_week:
            return None
        if (u % U) < U - 1:
            return (xbuf[u // U], ((u % U) + 1) * BL)
        if u < STEPS_PER_IT - 1:
            return (xbuf[1 - u // U], 0)
        return (xbuf[0], 0)

    with tile.TileContext(nc) as tc:
        # ---- preamble: weights, constants, state init ----
        nc.gpsimd.memset(c0.ap(), 0.0)
        nc.gpsimd.memset(c1f.ap(), 0.0)
        nc.sync.dma_start(out=whh0t.ap(), in_=whh0t_d.ap())
        nc.sync.dma_start(out=wih1t.ap(), in_=wih1t_d.ap())
        nc.sync.dma_start(out=whh1t.ap(), in_=whh1t_d.ap())
        nc.sync.dma_start(out=wih0t.ap(), in_=wih0t_d.ap())
        nc.sync.dma_start(out=bias1.ap(), in_=bias1_d.ap())
        nc.sync.dma_start(out=woutt.ap(), in_=woutt_d.ap())
        nc.sync.dma_start(out=ones.ap(), in_=ones_d.ap())
        nc.sync.dma_start(out=identc.ap(), in_=identc_d.ap())
        for p in (0, 1):
            nc.sync.dma_start(out=h0T[p].ap(), in_=zeros_d.ap())
            nc.sync.dma_start(out=h1T[p].ap(), in_=zeros_d.ap())
            nc.sync.dma_start(out=xbuf[p].ap()[:, 0:4 * BL], in_=zeros_d.ap())
            nc.sync.dma_start(out=xbuf[p].ap()[:, 4 * BL:8 * BL],
                              in_=zeros_d.ap())

        def week_loop(week, pred_week):
            blk_base = week * WEEK // U
            it_base = week * WEEK // STEPS_PER_IT

            def body(i):
                for a in (0, 1):
                    lo = 1 if pred_week else 0
                    nc.sync.dma_start(
                        out=xbuf[a].ap()[lo:KX, :],
                        in_=xaug_d.ap()[ds((blk_base + 2 * i + a) * KX + lo,
                                           KX - lo), :])
                run = lambda ts: [f() for f in ts]

                def step_head(u):
                    """transposes of h1(u-1), pred(u-1), staging writes —
                    runs right after EW1(u-1) completes. In original-data
                    weeks the next x-side matmuls fill the wait for the
                    second h1 half."""
                    up, parp = u - 1, (u - 1) % 2
                    emit_transpose1_half(parp, 0)
                    emit_pred_mms(up, (0, 1))
                    emit_transpose1_half(parp, 1)
                    emit_pred_mms(up, (2, 3))
                    emit_pred_writes(up, pred_week,
                                     pred_dst_for(up, pred_week))

                n_early, n_late = mm_whh0(0, True)
                run(n_early + n_late)
                _interleave(mm_xside(0, xbuf[0]), mm_bias1(0))
                for u in range(STEPS_PER_IT):
                    par = u % 2
                    emit_ew0_half(par, 0)       # ACT/DVE
                    emit_ew0_half(par, 1)
                    run(mm_whh1(u))             # PE fills the EW0 window
                    emit_transpose0_half(par, 0)
                    w_k01, w_k23a, w_k23b = mm_wih1(u)
                    if u + 1 < STEPS_PER_IT:
                        n_early, n_late = mm_whh0(u + 1, True)
                    else:
                        n_early, n_late = [], []
                    _interleave(w_k01, n_early)
                    emit_transpose0_half(par, 1)
                    run(w_k23a)
                    emit_ew1_half(par, 0)       # ACT/DVE
                    run(w_k23b)                 # gates1-h1 producers
                    emit_ew1_half(par, 1)
                    run(n_late)                 # fills the EW1 window
                    if u + 1 < STEPS_PER_IT:
                        step_head(u + 1)
                        _interleave(mm_xside(u + 1, xbuf[(u + 1) // U]),
                                    mm_bias1(u + 1))
                step_head(STEPS_PER_IT)
                nc.sync.dma_start(out=out_d.ap()[ds(it_base + i, 1), :],
                                  in_=outst.ap())

            with tc.For_i(0, IT_PER_WEEK, 1,
                          hint_engines=(mybir.EngineType.PE,
                                        mybir.EngineType.Activation,
                                        mybir.EngineType.DVE)) as i:
                body(i)

        week_loop(0, False)
        # pred(671) -> feat0 slot for t=672
        nc.vector.tensor_copy(xbuf[0].ap()[0:1, 0:BL], predl.ap())
        week_loop(1, True)
        week_loop(2, False)
        nc.vector.tensor_copy(xbuf[0].ap()[0:1, 0:BL], predl.ap())
        week_loop(3, True)

    nc.compile()
    return nc


def _prep_inputs(x, Wih0, Whh0, bih0, bhh0, Wih1, Whh1, bih1, bhh1, Wout, bout):
    """Host-side reshapes: gate reorder to [i,f,o,g], weight transposes,
    hi/lo bias split, per-core xaug staging layout. bf16 matmul operands."""
    f32 = np.float32
    bf16 = ml_dtypes.bfloat16
    HB = 256
    blocks = []
    for hb in range(2):
        blocks += [np.arange(0, 512)[hb*HB:(hb+1)*HB],          # i half
                   np.arange(512, 1024)[hb*HB:(hb+1)*HB],       # f half
                   np.arange(1536, 2048)[hb*HB:(hb+1)*HB],      # o half
                   np.arange(1024, 1536)[hb*HB:(hb+1)*HB]]      # g half
    perm = np.concatenate(blocks)

    def wT(w):  # [G, 512] -> [128, 4*G] chunk-k at cols [G*k, G*k+G)
        t = np.ascontiguousarray(w[perm].T.astype(f32))          # [512, G]
        return np.ascontiguousarray(
            t.reshape(4, 128, G).transpose(1, 0, 2).reshape(128, 4 * G)
        ).astype(bf16)

    def hilo(v):  # [G] fp32 -> [2, G] bf16 rows summing to ~v
        hi = v.astype(bf16).astype(f32)
        lo = (v - hi).astype(bf16)
        return np.stack([hi.astype(bf16), lo], axis=0)

    whh0t = wT(Whh0)
    wih1t = wT(Wih1)
    whh1t = wT(Whh1)
    bias0 = hilo((bih0 + bhh0)[perm].astype(f32))                # [2, G] bf16
    bias1 = np.zeros((128, G), bf16)
    bias1[0:2] = hilo((bih1 + bhh1)[perm].astype(f32))
    wih0p = Wih0[perm].astype(f32)                               # [G, 33]
    wih0t = np.zeros((128, G), bf16)
    wih0t[0:F + 1] = wih0p.T.astype(bf16)
    wih0t[F + 1:F + 3] = bias0
    woutt = np.ascontiguousarray(Wout.reshape(4, 128).T.astype(f32)).astype(bf16)

    onesp = np.zeros((128, BL), bf16)
    onesp[0:2] = 1.0

    tw = np.arange(S) // WEEK
    mask = np.where((tw % 2 == 0) & ((tw + 1) * WEEK <= S), 0.0, 1.0)
    flag = np.where((mask == 0.0) | (np.arange(S) == 0), 0.0, 1.0).astype(f32)

    xaugs = []
    for c in range(NCORES):
        xc = x[c * BL:(c + 1) * BL].astype(f32)        # [BL, S, F]
        arr = np.empty((S, KX, BL), f32)
        arr[:, 0, :] = xc[:, :, 0].T
        arr[:, 1:F, :] = xc[:, :, 1:].transpose(1, 2, 0)
        arr[:, F, :] = flag[:, None]
        arr[:, F + 1, :] = 1.0
        arr[:, F + 2, :] = 1.0
        a = arr.reshape(NBLK, U, KX, BL).transpose(0, 2, 1, 3)
        xaugs.append(np.ascontiguousarray(
            a.reshape(NBLK * KX, U * BL)).astype(bf16))

    shared = {
        "whh0t": whh0t, "wih1t": wih1t, "whh1t": whh1t, "wih0t": wih0t,
        "bias1": np.ascontiguousarray(bias1), "woutt": woutt,
        "ones": onesp, "zeros": np.zeros((128, 4 * BL), bf16),
        "identc": np.ascontiguousarray(
            np.tile(np.eye(BL, dtype=f32), (2, 1))).astype(bf16),
    }
    in_maps = [dict(shared, xaug=xaugs[c]) for c in range(NCORES)]
    return in_maps, float(np.asarray(bout).reshape(-1)[0])


def kernel(x, Wih0, Whh0, bih0, bhh0, Wih1, Whh1, bih1, bhh1, Wout, bout,
           _trace=False):
    global _LAST_RESULTS
    x = np.asarray(x)
    in_maps, bout_val = _prep_inputs(
        x, np.asarray(Wih0), np.asarray(Whh0), np.asarray(bih0),
        np.asarray(bhh0), np.asarray(Wih1), np.asarray(Whh1),
        np.asarray(bih1), np.asarray(bhh1), np.asarray(Wout),
        np.asarray(bout))
    nc = _build(bout_val, trace=_trace)
    res = run_bass_kernel_spmd(nc, in_maps, core_ids=list(range(NCORES)),
                               trace=_trace)
    _LAST_RESULTS = res
    out = np.empty((B, S, 1), np.float32)
    for c in range(NCORES):
        oc = res.results[c]["out"].reshape(S, BL)     # [S, BL]
        out[c * BL:(c + 1) * BL, :, 0] = oc.T
    return out


# revision 19
# speedup vs baseline: 1.2251x; 1.0043x over previous
"""Trainium2 Bass kernel for nn_AlternatingForecastModel.

2-layer LSTM (H=512) over S=2688 steps, B=512. Odd weeks feed the model's
previous prediction back as input feature 0. Data-parallel over batch:
8 cores x 64 rows, weights replicated, scan local per core.

Per core/step (bl=64): gates[bl, 2048] accumulate in PSUM via bf16 matmuls
with lhsT = transposed activations (curT [35,64] / hT chunks [128,64]) and
rhs = pre-transposed weights streamed at N=512. Layer-0 gates live in PSUM
partitions 0-63 (banks 0-3), layer-1 gates in partitions 64-127 (banks 4-7):
matmuls of the two layers target disjoint PE column groups and execute
concurrently (col tiling), with emission interleaved to pair them. Gate rows
are host-reordered to [i, f, o, g] so one sigmoid covers cols 0:1536.
Biases enter as hi+lo bf16 ones-rows (exact to ~2^-17). Elementwise (fp32)
on ACT/DVE; h_new transposed back via PE transposes into reused PSUM banks,
evacuated as bf16; pred = wout . h1 via M=1 matmuls giving predT [1, 64].
Emission software-pipelines: gates1's h1-part pairs with gates0, the next
step's Whh0-part pairs with gates1's h0-part.
"""

import numpy as np
import ml_dtypes

import concourse.bacc as bacc
import concourse.mybir as mybir
import concourse.tile as tile
from concourse.bass import ds
from concourse.bass_utils import run_bass_kernel_spmd

FP32 = mybir.dt.float32
BF16 = mybir.dt.bfloat16
AF = mybir.ActivationFunctionType

B, S, F = 512, 2688, 32
H = 512
G = 4 * H
WEEK = 672
NCORES = 8
BL = B // NCORES          # 64 batch rows per core
KX = F + 3                # 35: [feat0, x1..x31, flag, ones_hi, ones_lo]
U = 8                     # steps per sub-block (one x DMA)
STEPS_PER_IT = 2 * U      # 16
IT_PER_WEEK = WEEK // STEPS_PER_IT  # 42
NBLK = S // U             # 336
NIT = S // STEPS_PER_IT   # 168

_LAST_RESULTS = None


def _interleave(la, lb):
    """Alternate emission of two thunk lists (A/B PE col groups)."""
    n = max(len(la), len(lb))
    for i in range(n):
        if i < len(lb):
            lb[i]()
        if i < len(la):
            la[i]()


def _build(bout_val: float, trace: bool = False):
    nc = bacc.Bacc("TRN2")

    xaug_d = nc.declare_dram_parameter("xaug", [NBLK * KX, U * BL], BF16, isOutput=False)
    whh0t_d = nc.declare_dram_parameter("whh0t", [128, 4 * G], BF16, isOutput=False)
    wih1t_d = nc.declare_dram_parameter("wih1t", [128, 4 * G], BF16, isOutput=False)
    whh1t_d = nc.declare_dram_parameter("whh1t", [128, 4 * G], BF16, isOutput=False)
    wih0t_d = nc.declare_dram_parameter("wih0t", [128, G], BF16, isOutput=False)
    bias1_d = nc.declare_dram_parameter("bias1", [128, G], BF16, isOutput=False)
    woutt_d = nc.declare_dram_parameter("woutt", [128, 4], BF16, isOutput=False)
    ones_d = nc.declare_dram_parameter("ones", [128, BL], BF16, isOutput=False)
    zeros_d = nc.declare_dram_parameter("zeros", [128, 4 * BL], BF16, isOutput=False)
    identc_d = nc.declare_dram_parameter("identc", [128, BL], BF16, isOutput=False)
    out_d = nc.declare_dram_parameter("out", [NIT, STEPS_PER_IT * BL], FP32, isOutput=True)

    # SBUF
    whh0t = nc.alloc_sbuf_tensor("whh0t_s", [128, 4 * G], BF16)
    wih1t = nc.alloc_sbuf_tensor("wih1t_s", [128, 4 * G], BF16)
    whh1t = nc.alloc_sbuf_tensor("whh1t_s", [128, 4 * G], BF16)
    wih0t = nc.alloc_sbuf_tensor("wih0t_s", [128, G], BF16)
    bias1 = nc.alloc_sbuf_tensor("bias1_s", [128, G], BF16)
    woutt = nc.alloc_sbuf_tensor("woutt_s", [128, 4], BF16)
    ones = nc.alloc_sbuf_tensor("ones_s", [128, BL], BF16)
    identc = nc.alloc_sbuf_tensor("identc_s", [128, BL], BF16)

    xbuf = [nc.alloc_sbuf_tensor(f"xbuf{a}", [128, U * BL], BF16) for a in (0, 1)]
    h0T = [nc.alloc_sbuf_tensor(f"h0T{p}", [128, 4 * BL], BF16) for p in (0, 1)]
    h1T = [nc.alloc_sbuf_tensor(f"h1T{p}", [128, 4 * BL], BF16) for p in (0, 1)]
    # layer-0 elementwise state at partitions 0-63, layer-1 at 64-127
    c0 = nc.alloc_sbuf_tensor("c0", [BL, H], FP32)
    c1f = nc.alloc_sbuf_tensor("c1f", [128, H], FP32)
    sig0 = [nc.alloc_sbuf_tensor(f"sig0{p}", [BL, 3 * H], FP32) for p in (0, 1)]
    sig1 = [nc.alloc_sbuf_tensor(f"sig1{p}", [128, 3 * H], FP32) for p in (0, 1)]
    tg0 = [nc.alloc_sbuf_tensor(f"tg0{p}", [BL, H], FP32) for p in (0, 1)]
    tg1 = [nc.alloc_sbuf_tensor(f"tg1{p}", [128, H], FP32) for p in (0, 1)]
    tc0 = [nc.alloc_sbuf_tensor(f"tc0{p}", [BL, H], FP32) for p in (0, 1)]
    tc1 = [nc.alloc_sbuf_tensor(f"tc1{p}", [128, H], FP32) for p in (0, 1)]
    ta0 = [nc.alloc_sbuf_tensor(f"ta0{p}", [BL, H], FP32) for p in (0, 1)]
    tb0 = [nc.alloc_sbuf_tensor(f"tb0{p}", [BL, H], FP32) for p in (0, 1)]
    ta1 = [nc.alloc_sbuf_tensor(f"ta1{p}", [128, H], FP32) for p in (0, 1)]
    tb1 = [nc.alloc_sbuf_tensor(f"tb1{p}", [128, H], FP32) for p in (0, 1)]
    h0 = [nc.alloc_sbuf_tensor(f"h0{p}", [BL, H], BF16) for p in (0, 1)]
    h1 = [nc.alloc_sbuf_tensor(f"h1{p}", [128, H], BF16) for p in (0, 1)]
    outst = nc.alloc_sbuf_tensor("outst", [1, STEPS_PER_IT * BL], FP32)
    predl = nc.alloc_sbuf_tensor("predl", [1, BL], BF16)

    g0 = nc.alloc_psum_tensor("g0", [128, G], FP32)
    g1 = nc.alloc_psum_tensor("g1", [128, G], FP32)

    NS = G // 512  # 4 N-slices per gate vector

    # ---- thunk builders for PE matmul groups (A = layer0/parts 0-63,
    #      B = layer1/parts 64-127) ----

    def mm_whh0(u, first):
        """Whh0-part of gates0(t) emitted during step t-1. `early` (bank 1,
        chunks 0-1) may interleave mid-step; `late` (incl. all bank-0 MMs)
        must follow transpose0-half1 + CASTs, since bank 0 bytes 0-511 are
        the transpose scratch. start=True on the first write of each bank."""
        prev = (u + 1) % 2

        def f(k, ns):
            lhs = h0T[prev].ap()[:, k * BL:(k + 1) * BL]
            def g(k=k, ns=ns, lhs=lhs):
                nc.tensor.matmul(
                    g0.ap()[0:BL, ns * 512:(ns + 1) * 512],
                    lhs,
                    whh0t.ap()[:, k * G + ns * 512:k * G + (ns + 1) * 512],
                    start=(first and k == 0), stop=False)
            return g
        early = [f(0, 1), f(1, 1)]
        late = ([f(k, 0) for k in range(4)]
                + [f(k, ns) for ns in (2, 3) for k in (0, 1)]
                + [f(k, ns) for ns in (1, 2, 3) for k in (2, 3)])
        return early, late

    def mm_xside(u, xb):
        slot = (u % U) * BL
        xl = xb.ap()[0:128, slot:slot + BL]
        out = []
        for ns in range(NS):
            def f(ns=ns):
                nc.tensor.matmul(g0.ap()[0:BL, ns * 512:(ns + 1) * 512],
                                 xl, wih0t.ap()[:, ns * 512:(ns + 1) * 512],
                                 start=False, stop=(ns == NS - 1))
            out.append(f)
        return out

    def mm_bias1(u):
        out = []
        for ns in range(NS):
            def f(ns=ns):
                nc.tensor.matmul(g1.ap()[64:128, ns * 512:(ns + 1) * 512],
                                 ones.ap()[0:128, :],
                                 bias1.ap()[:, ns * 512:(ns + 1) * 512],
                                 start=True, stop=False)
            out.append(f)
        return out

    def mm_whh1(u):
        prev = (u + 1) % 2
        out = []
        for k in range(4):
            lhs = h1T[prev].ap()[:, k * BL:(k + 1) * BL]
            for ns in range(NS):
                def f(k=k, ns=ns, lhs=lhs):
                    nc.tensor.matmul(
                        g1.ap()[64:128, ns * 512:(ns + 1) * 512],
                        lhs,
                        whh1t.ap()[:, k * G + ns * 512:k * G + (ns + 1) * 512],
                        start=False, stop=False)
                out.append(f)
        return out

    def mm_wih1(u):
        """Wih1-part of gates1(t). k01: ns-major over chunks 0-1 (gated on
        CAST-h0a); k23a: ns0-1 of chunks 2-3 (sig1-h0 gate); k23b: rest."""
        par = u % 2

        def f(k, ns):
            lhs = h0T[par].ap()[:, k * BL:(k + 1) * BL]
            def g(k=k, ns=ns, lhs=lhs):
                nc.tensor.matmul(
                    g1.ap()[64:128, ns * 512:(ns + 1) * 512],
                    lhs,
                    wih1t.ap()[:, k * G + ns * 512:k * G + (ns + 1) * 512],
                    start=False, stop=(k == 3 and ns == 3))
            return g
        k01 = [f(k, ns) for ns in range(NS) for k in (0, 1)]
        k23a = [f(k, ns) for ns in (0, 1) for k in (2, 3)]
        k23b = [f(k, ns) for ns in (2, 3) for k in (2, 3)]
        return k01, k23a, k23b

    # ---- elementwise ----

    HB = 256  # hidden half-block; gate rows are [i,f,o,g]x2 halves of 1024

    g0b = g0.bitcast(BF16)   # [128, 4096] bf16 view for transpose targets
    g1b = g1.bitcast(BF16)

    def emit_ew0_half(par, hb):
        """Layer-0 elementwise for one hidden half."""
        sg = sig0[par].ap()
        if True:
            gofs = hb * 1024
            nc.scalar.activation(sg[:, hb * 768:hb * 768 + 768],
                                 g0.ap()[0:BL, gofs:gofs + 768], AF.Sigmoid)
            nc.scalar.activation(tg0[par].ap()[:, hb * HB:hb * HB + HB],
                                 g0.ap()[0:BL, gofs + 768:gofs + 1024], AF.Tanh)
            nc.vector.tensor_mul(ta0[par].ap()[:, hb * HB:hb * HB + HB],
                                 sg[:, hb * 768:hb * 768 + HB],
                                 tg0[par].ap()[:, hb * HB:hb * HB + HB])
            nc.vector.tensor_mul(tb0[par].ap()[:, hb * HB:hb * HB + HB],
                                 sg[:, hb * 768 + HB:hb * 768 + 2 * HB],
                                 c0.ap()[:, hb * HB:hb * HB + HB])
            nc.vector.tensor_add(c0.ap()[:, hb * HB:hb * HB + HB],
                                 ta0[par].ap()[:, hb * HB:hb * HB + HB],
                                 tb0[par].ap()[:, hb * HB:hb * HB + HB])
            nc.scalar.activation(tc0[par].ap()[:, hb * HB:hb * HB + HB],
                                 c0.ap()[:, hb * HB:hb * HB + HB], AF.Tanh)
            nc.vector.tensor_mul(h0[par].ap()[:, hb * HB:hb * HB + HB],
                                 sg[:, hb * 768 + 2 * HB:hb * 768 + 3 * HB],
                                 tc0[par].ap()[:, hb * HB:hb * HB + HB])

    def emit_transpose0_half(par, hb):
        for k in (2 * hb, 2 * hb + 1):
            nc.tensor.transpose(g0b.ap()[0:128, k * BL:(k + 1) * BL],
                                h0[par].ap()[0:BL, k * 128:(k + 1) * 128],
                                identc.ap()[0:BL, :])
        nc.vector.tensor_copy(
            h0T[par].ap()[:, hb * 2 * BL:(hb + 1) * 2 * BL],
            g0b.ap()[0:128, hb * 2 * BL:(hb + 1) * 2 * BL])

    def emit_ew1_half(par, hb):
        sg = sig1[par].ap()[64:128, :]
        tg = tg1[par].ap()[64:128, :]
        cc = c1f.ap()[64:128, :]
        aa = ta1[par].ap()[64:128, :]
        bb = tb1[par].ap()[64:128, :]
        tcc = tc1[par].ap()[64:128, :]
        hh = h1[par].ap()[64:128, :]
        if True:
            gofs = hb * 1024
            nc.scalar.activation(sg[:, hb * 768:hb * 768 + 768],
                                 g1.ap()[64:128, gofs:gofs + 768], AF.Sigmoid)
            nc.scalar.activation(tg[:, hb * HB:hb * HB + HB],
                                 g1.ap()[64:128, gofs + 768:gofs + 1024],
                                 AF.Tanh)
            nc.vector.tensor_mul(aa[:, hb * HB:hb * HB + HB],
                                 sg[:, hb * 768:hb * 768 + HB],
                                 tg[:, hb * HB:hb * HB + HB])
            nc.vector.tensor_mul(bb[:, hb * HB:hb * HB + HB],
                                 sg[:, hb * 768 + HB:hb * 768 + 2 * HB],
                                 cc[:, hb * HB:hb * HB + HB])
            nc.vector.tensor_add(cc[:, hb * HB:hb * HB + HB],
                                 aa[:, hb * HB:hb * HB + HB],
                                 bb[:, hb * HB:hb * HB + HB])
            nc.scalar.activation(tcc[:, hb * HB:hb * HB + HB],
                                 cc[:, hb * HB:hb * HB + HB], AF.Tanh)
            nc.vector.tensor_mul(hh[:, hb * HB:hb * HB + HB],
                                 sg[:, hb * 768 + 2 * HB:hb * 768 + 3 * HB],
                                 tcc[:, hb * HB:hb * HB + HB])

    def emit_transpose1_half(par, hb):
        for k in (2 * hb, 2 * hb + 1):
            nc.tensor.transpose(g1b.ap()[0:128, k * BL:(k + 1) * BL],
                                h1[par].ap()[64:128, k * 128:(k + 1) * 128],
                                identc.ap()[64:128, :])
        nc.vector.tensor_copy(
            h1T[par].ap()[:, hb * 2 * BL:(hb + 1) * 2 * BL],
            g1b.ap()[0:128, hb * 2 * BL:(hb + 1) * 2 * BL])

    def emit_pred_mms(u, ks):
        par = u % 2
        pps = g1.ap()[0:1, 512:512 + BL]
        for k in ks:
            nc.tensor.matmul(pps, woutt.ap()[:, k:k + 1],
                             h1T[par].ap()[:, k * BL:(k + 1) * BL],
                             start=(k == 0), stop=(k == 3))

    def emit_pred_writes(u, pred_week, pred_dst):
        pps = g1.ap()[0:1, 512:512 + BL]
        nc.vector.tensor_scalar_add(outst.ap()[0:1, u * BL:(u + 1) * BL], pps,
                                    bout_val)
        if pred_week:
            dst_t, dst_col = pred_dst
            nc.vector.tensor_scalar_add(dst_t.ap()[0:1, dst_col:dst_col + BL],
                                        pps, bout_val)
        else:
            nc.vector.tensor_scalar_add(predl.ap(), pps, bout_val)

    def pred_dst_for(u, pred_week):
        if not pred_week:
            return None
        if (u % U) < U - 1:
            return (xbuf[u // U], ((u % U) + 1) * BL)
        if u < STEPS_PER_IT - 1:
            return (xbuf[1 - u // U], 0)
        return (xbuf[0], 0)

    with tile.TileContext(nc) as tc:
        # ---- preamble: weights, constants, state init ----
        nc.gpsimd.memset(c0.ap(), 0.0)
        nc.gpsimd.memset(c1f.ap(), 0.0)
        nc.sync.dma_start(out=whh0t.ap(), in_=whh0t_d.ap())
        nc.sync.dma_start(out=wih1t.ap(), in_=wih1t_d.ap())
        nc.sync.dma_start(out=whh1t.ap(), in_=whh1t_d.ap())
        nc.sync.dma_start(out=wih0t.ap(), in_=wih0t_d.ap())
        nc.sync.dma_start(out=bias1.ap(), in_=bias1_d.ap())
        nc.sync.dma_start(out=woutt.ap(), in_=woutt_d.ap())
        nc.sync.dma_start(out=ones.ap(), in_=ones_d.ap())
        nc.sync.dma_start(out=identc.ap(), in_=identc_d.ap())
        for p in (0, 1):
            nc.sync.dma_start(out=h0T[p].ap(), in_=zeros_d.ap())
            nc.sync.dma_start(out=h1T[p].ap(), in_=zeros_d.ap())
            nc.sync.dma_start(out=xbuf[p].ap()[:, 0:4 * BL], in_=zeros_d.ap())
            nc.sync.dma_start(out=xbuf[p].ap()[:, 4 * BL:8 * BL],
                              in_=zeros_d.ap())

        def week_loop(week, pred_week):
            blk_base = week * WEEK // U
            it_base = week * WEEK // STEPS_PER_IT

            def body(i):
                for a in (0, 1):
                    lo = 1 if pred_week else 0
                    nc.sync.dma_start(
                        out=xbuf[a].ap()[lo:KX, :],
                        in_=xaug_d.ap()[ds((blk_base + 2 * i + a) * KX + lo,
                                           KX - lo), :])
                run = lambda ts: [f() for f in ts]

                def step_head(u):
                    """transposes of h1(u-1), pred(u-1), staging writes —
                    runs right after EW1(u-1) completes. In original-data
                    weeks the next x-side matmuls fill the wait for the
                    second h1 half."""
                    up, parp = u - 1, (u - 1) % 2
                    emit_transpose1_half(parp, 0)
                    emit_pred_mms(up, (0, 1))
                    emit_transpose1_half(parp, 1)
                    emit_pred_mms(up, (2, 3))
                    emit_pred_writes(up, pred_week,
                                     pred_dst_for(up, pred_week))

                n_early, n_late = mm_whh0(0, True)
                run(n_early + n_late)
                _interleave(mm_xside(0, xbuf[0]), mm_bias1(0))
                for u in range(STEPS_PER_IT):
                    par = u % 2
                    emit_ew0_half(par, 0)       # ACT/DVE
                    emit_ew0_half(par, 1)
                    run(mm_whh1(u))             # PE fills the EW0 window
                    emit_transpose0_half(par, 0)
                    w_k01, w_k23a, w_k23b = mm_wih1(u)
                    if u + 1 < STEPS_PER_IT:
                        n_early, n_late = mm_whh0(u + 1, True)
                    else:
                        n_early, n_late = [], []
                    _interleave(w_k01, n_early)
                    emit_transpose0_half(par, 1)
                    run(w_k23a)
                    emit_ew1_half(par, 0)       # ACT/DVE
                    run(w_k23b)                 # gates1-h1 producers
                    emit_ew1_half(par, 1)
                    run(n_late)                 # fills the EW1 window
                    if u + 1 < STEPS_PER_IT:
                        step_head(u + 1)
                        _interleave(mm_xside(u + 1, xbuf[(u + 1) // U]),
                                    mm_bias1(u + 1))
                step_head(STEPS_PER_IT)
                nc.sync.dma_start(out=out_d.ap()[ds(it_base + i, 1), :],
                                  in_=outst.ap())

            with tc.For_i(0, IT_PER_WEEK, 1, staggered_reset=True,
                          hint_engines=(mybir.EngineType.PE,
                                        mybir.EngineType.Activation,
                                        mybir.EngineType.DVE)) as i:
                body(i)

        week_loop(0, False)
        # pred(671) -> feat0 slot for t=672
        nc.vector.tensor_copy(xbuf[0].ap()[0:1, 0:BL], predl.ap())
        week_loop(1, True)
        week_loop(2, False)
        nc.vector.tensor_copy(xbuf[0].ap()[0:1, 0:BL], predl.ap())
        week_loop(3, True)

    nc.compile()
    return nc


def _prep_inputs(x, Wih0, Whh0, bih0, bhh0, Wih1, Whh1, bih1, bhh1, Wout, bout):
    """Host-side reshapes: gate reorder to [i,f,o,g], weight transposes,
    hi/lo bias split, per-core xaug staging layout. bf16 matmul operands."""
    f32 = np.float32
    bf16 = ml_dtypes.bfloat16
    HB = 256
    blocks = []
    for hb in range(2):
        blocks += [np.arange(0, 512)[hb*HB:(hb+1)*HB],          # i half
                   np.arange(512, 1024)[hb*HB:(hb+1)*HB],       # f half
                   np.arange(1536, 2048)[hb*HB:(hb+1)*HB],      # o half
                   np.arange(1024, 1536)[hb*HB:(hb+1)*HB]]      # g half
    perm = np.concatenate(blocks)

    def wT(w):  # [G, 512] -> [128, 4*G] chunk-k at cols [G*k, G*k+G)
        t = np.ascontiguousarray(w[perm].T.astype(f32))          # [512, G]
        return np.ascontiguousarray(
            t.reshape(4, 128, G).transpose(1, 0, 2).reshape(128, 4 * G)
        ).astype(bf16)

    def hilo(v):  # [G] fp32 -> [2, G] bf16 rows summing to ~v
        hi = v.astype(bf16).astype(f32)
        lo = (v - hi).astype(bf16)
        return np.stack([hi.astype(bf16), lo], axis=0)

    whh0t = wT(Whh0)
    wih1t = wT(Wih1)
    whh1t = wT(Whh1)
    bias0 = hilo((bih0 + bhh0)[perm].astype(f32))                # [2, G] bf16
    bias1 = np.zeros((128, G), bf16)
    bias1[0:2] = hilo((bih1 + bhh1)[perm].astype(f32))
    wih0p = Wih0[perm].astype(f32)                               # [G, 33]
    wih0t = np.zeros((128, G), bf16)
    wih0t[0:F + 1] = wih0p.T.astype(bf16)
    wih0t[F + 1:F + 3] = bias0
    woutt = np.ascontiguousarray(Wout.reshape(4, 128).T.astype(f32)).astype(bf16)

    onesp = np.zeros((128, BL), bf16)
    onesp[0:2] = 1.0

    tw = np.arange(S) // WEEK
    mask = np.where((tw % 2 == 0) & ((tw + 1) * WEEK <= S), 0.0, 1.0)
    flag = np.where((mask == 0.0) | (np.arange(S) == 0), 0.0, 1.0).astype(f32)

    xaugs = []
    for c in range(NCORES):
        xc = x[c * BL:(c + 1) * BL].astype(f32)        # [BL, S, F]
        arr = np.empty((S, KX, BL), f32)
        arr[:, 0, :] = xc[:, :, 0].T
        arr[:, 1:F, :] = xc[:, :, 1:].transpose(1, 2, 0)
        arr[:, F, :] = flag[:, None]
        arr[:, F + 1, :] = 1.0
        arr[:, F + 2, :] = 1.0
        a = arr.reshape(NBLK, U, KX, BL).transpose(0, 2, 1, 3)
        xaugs.append(np.ascontiguousarray(
            a.reshape(NBLK * KX, U * BL)).astype(bf16))

    shared = {
        "whh0t": whh0t, "wih1t": wih1t, "whh1t": whh1t, "wih0t": wih0t,
        "bias1": np.ascontiguousarray(bias1), "woutt": woutt,
        "ones": onesp, "zeros": np.zeros((128, 4 * BL), bf16),
        "identc": np.ascontiguousarray(
            np.tile(np.eye(BL, dtype=f32), (2, 1))).astype(bf16),
    }
    in_maps = [dict(shared, xaug=xaugs[c]) for c in range(NCORES)]
    return in_maps, float(np.asarray(bout).reshape(-1)[0])


def kernel(x, Wih0, Whh0, bih0, bhh0, Wih1, Whh1, bih1, bhh1, Wout, bout,
           _trace=False):
    global _LAST_RESULTS
    x = np.asarray(x)
    in_maps, bout_val = _prep_inputs(
        x, np.asarray(Wih0), np.asarray(Whh0), np.asarray(bih0),
        np.asarray(bhh0), np.asarray(Wih1), np.asarray(Whh1),
        np.asarray(bih1), np.asarray(bhh1), np.asarray(Wout),
        np.asarray(bout))
    nc = _build(bout_val, trace=_trace)
    res = run_bass_kernel_spmd(nc, in_maps, core_ids=list(range(NCORES)),
                               trace=_trace)
    _LAST_RESULTS = res
    out = np.empty((B, S, 1), np.float32)
    for c in range(NCORES):
        oc = res.results[c]["out"].reshape(S, BL)     # [S, BL]
        out[c * BL:(c + 1) * BL, :, 0] = oc.T
    return out


# revision 21
# speedup vs baseline: 1.2351x; 1.0081x over previous
"""Trainium2 Bass kernel for nn_AlternatingForecastModel.

2-layer LSTM (H=512) over S=2688 steps, B=512. Odd weeks feed the model's
previous prediction back as input feature 0. Data-parallel over batch:
8 cores x 64 rows, weights replicated, scan local per core.

Per core/step (bl=64): gates[bl, 2048] accumulate in PSUM via bf16 matmuls
with lhsT = transposed activations (curT [35,64] / hT chunks [128,64]) and
rhs = pre-transposed weights streamed at N=512. Layer-0 gates live in PSUM
partitions 0-63 (banks 0-3), layer-1 gates in partitions 64-127 (banks 4-7):
matmuls of the two layers target disjoint PE column groups and execute
concurrently (col tiling), with emission interleaved to pair them. Gate rows
are host-reordered to [i, f, o, g] so one sigmoid covers cols 0:1536.
Biases enter as hi+lo bf16 ones-rows (exact to ~2^-17). Elementwise (fp32)
on ACT/DVE; h_new transposed back via PE transposes into reused PSUM banks,
evacuated as bf16; pred = wout . h1 via M=1 matmuls giving predT [1, 64].
Emission software-pipelines: gates1's h1-part pairs with gates0, the next
step's Whh0-part pairs with gates1's h0-part.
"""

import numpy as np
import ml_dtypes

import concourse.bacc as bacc
import concourse.mybir as mybir
import concourse.tile as tile
from concourse.bass import ds
from concourse.bass_utils import run_bass_kernel_spmd

FP32 = mybir.dt.float32
BF16 = mybir.dt.bfloat16
AF = mybir.ActivationFunctionType

B, S, F = 512, 2688, 32
H = 512
G = 4 * H
WEEK = 672
NCORES = 8
BL = B // NCORES          # 64 batch rows per core
KX = F + 3                # 35: [feat0, x1..x31, flag, ones_hi, ones_lo]
U = 8                     # steps per sub-block (one x DMA)
SUBS_PER_IT = 4
STEPS_PER_IT = SUBS_PER_IT * U  # 32
IT_PER_WEEK = WEEK // STEPS_PER_IT  # 42
NBLK = S // U             # 336
NIT = S // STEPS_PER_IT   # 168

_LAST_RESULTS = None


def _interleave(la, lb):
    """Alternate emission of two thunk lists (A/B PE col groups)."""
    n = max(len(la), len(lb))
    for i in range(n):
        if i < len(lb):
            lb[i]()
        if i < len(la):
            la[i]()


def _build(bout_val: float, trace: bool = False):
    nc = bacc.Bacc("TRN2")

    xaug_d = nc.declare_dram_parameter("xaug", [NBLK * KX, U * BL], BF16, isOutput=False)
    whh0t_d = nc.declare_dram_parameter("whh0t", [128, 4 * G], BF16, isOutput=False)
    wih1t_d = nc.declare_dram_parameter("wih1t", [128, 4 * G], BF16, isOutput=False)
    whh1t_d = nc.declare_dram_parameter("whh1t", [128, 4 * G], BF16, isOutput=False)
    wih0t_d = nc.declare_dram_parameter("wih0t", [128, G], BF16, isOutput=False)
    bias1_d = nc.declare_dram_parameter("bias1", [128, G], BF16, isOutput=False)
    woutt_d = nc.declare_dram_parameter("woutt", [128, 4], BF16, isOutput=False)
    ones_d = nc.declare_dram_parameter("ones", [128, BL], BF16, isOutput=False)
    zeros_d = nc.declare_dram_parameter("zeros", [128, 4 * BL], BF16, isOutput=False)
    identc_d = nc.declare_dram_parameter("identc", [128, BL], BF16, isOutput=False)
    out_d = nc.declare_dram_parameter("out", [NIT, STEPS_PER_IT * BL], FP32, isOutput=True)

    # SBUF
    whh0t = nc.alloc_sbuf_tensor("whh0t_s", [128, 4 * G], BF16)
    wih1t = nc.alloc_sbuf_tensor("wih1t_s", [128, 4 * G], BF16)
    whh1t = nc.alloc_sbuf_tensor("whh1t_s", [128, 4 * G], BF16)
    wih0t = nc.alloc_sbuf_tensor("wih0t_s", [128, G], BF16)
    bias1 = nc.alloc_sbuf_tensor("bias1_s", [128, G], BF16)
    woutt = nc.alloc_sbuf_tensor("woutt_s", [128, 4], BF16)
    ones = nc.alloc_sbuf_tensor("ones_s", [128, BL], BF16)
    identc = nc.alloc_sbuf_tensor("identc_s", [128, BL], BF16)

    xbuf = [nc.alloc_sbuf_tensor(f"xbuf{a}", [128, U * BL], BF16) for a in (0, 1)]
    h0T = [nc.alloc_sbuf_tensor(f"h0T{p}", [128, 4 * BL], BF16) for p in (0, 1)]
    h1T = [nc.alloc_sbuf_tensor(f"h1T{p}", [128, 4 * BL], BF16) for p in (0, 1)]
    # layer-0 elementwise state at partitions 0-63, layer-1 at 64-127
    c0 = nc.alloc_sbuf_tensor("c0", [BL, H], FP32)
    c1f = nc.alloc_sbuf_tensor("c1f", [128, H], FP32)
    sig0 = [nc.alloc_sbuf_tensor(f"sig0{p}", [BL, 3 * H], FP32) for p in (0, 1)]
    sig1 = [nc.alloc_sbuf_tensor(f"sig1{p}", [128, 3 * H], FP32) for p in (0, 1)]
    tg0 = [nc.alloc_sbuf_tensor(f"tg0{p}", [BL, H], FP32) for p in (0, 1)]
    tg1 = [nc.alloc_sbuf_tensor(f"tg1{p}", [128, H], FP32) for p in (0, 1)]
    tc0 = [nc.alloc_sbuf_tensor(f"tc0{p}", [BL, H], FP32) for p in (0, 1)]
    tc1 = [nc.alloc_sbuf_tensor(f"tc1{p}", [128, H], FP32) for p in (0, 1)]
    ta0 = [nc.alloc_sbuf_tensor(f"ta0{p}", [BL, H], FP32) for p in (0, 1)]
    tb0 = [nc.alloc_sbuf_tensor(f"tb0{p}", [BL, H], FP32) for p in (0, 1)]
    ta1 = [nc.alloc_sbuf_tensor(f"ta1{p}", [128, H], FP32) for p in (0, 1)]
    tb1 = [nc.alloc_sbuf_tensor(f"tb1{p}", [128, H], FP32) for p in (0, 1)]
    h0 = [nc.alloc_sbuf_tensor(f"h0{p}", [BL, H], BF16) for p in (0, 1)]
    h1 = [nc.alloc_sbuf_tensor(f"h1{p}", [128, H], BF16) for p in (0, 1)]
    outst = nc.alloc_sbuf_tensor("outst", [1, STEPS_PER_IT * BL], FP32)
    predl = nc.alloc_sbuf_tensor("predl", [1, BL], BF16)

    g0 = nc.alloc_psum_tensor("g0", [128, G], FP32)
    g1 = nc.alloc_psum_tensor("g1", [128, G], FP32)

    NS = G // 512  # 4 N-slices per gate vector

    # ---- thunk builders for PE matmul groups (A = layer0/parts 0-63,
    #      B = layer1/parts 64-127) ----

    def mm_whh0(u, first):
        """Whh0-part of gates0(t) emitted during step t-1. `early` (bank 1,
        chunks 0-1) may interleave mid-step; `late` (incl. all bank-0 MMs)
        must follow transpose0-half1 + CASTs, since bank 0 bytes 0-511 are
        the transpose scratch. start=True on the first write of each bank."""
        prev = (u + 1) % 2

        def f(k, ns):
            lhs = h0T[prev].ap()[:, k * BL:(k + 1) * BL]
            def g(k=k, ns=ns, lhs=lhs):
                nc.tensor.matmul(
                    g0.ap()[0:BL, ns * 512:(ns + 1) * 512],
                    lhs,
                    whh0t.ap()[:, k * G + ns * 512:k * G + (ns + 1) * 512],
                    start=(first and k == 0), stop=False)
            return g
        early = [f(0, 1), f(1, 1)]
        late = ([f(k, 0) for k in range(4)]
                + [f(k, ns) for ns in (2, 3) for k in (0, 1)]
                + [f(k, ns) for ns in (1, 2, 3) for k in (2, 3)])
        return early, late

    def mm_xside(u, xb):
        slot = (u % U) * BL
        xl = xb.ap()[0:128, slot:slot + BL]
        out = []
        for ns in range(NS):
            def f(ns=ns):
                nc.tensor.matmul(g0.ap()[0:BL, ns * 512:(ns + 1) * 512],
                                 xl, wih0t.ap()[:, ns * 512:(ns + 1) * 512],
                                 start=False, stop=(ns == NS - 1))
            out.append(f)
        return out

    def mm_bias1(u):
        out = []
        for ns in range(NS):
            def f(ns=ns):
                nc.tensor.matmul(g1.ap()[64:128, ns * 512:(ns + 1) * 512],
                                 ones.ap()[0:128, :],
                                 bias1.ap()[:, ns * 512:(ns + 1) * 512],
                                 start=True, stop=False)
            out.append(f)
        return out

    def mm_whh1(u):
        prev = (u + 1) % 2
        out = []
        for k in range(4):
            lhs = h1T[prev].ap()[:, k * BL:(k + 1) * BL]
            for ns in range(NS):
                def f(k=k, ns=ns, lhs=lhs):
                    nc.tensor.matmul(
                        g1.ap()[64:128, ns * 512:(ns + 1) * 512],
                        lhs,
                        whh1t.ap()[:, k * G + ns * 512:k * G + (ns + 1) * 512],
                        start=False, stop=False)
                out.append(f)
        return out

    def mm_wih1(u):
        """Wih1-part of gates1(t). k01: ns-major over chunks 0-1 (gated on
        CAST-h0a); k23a: ns0-1 of chunks 2-3 (sig1-h0 gate); k23b: rest."""
        par = u % 2

        def f(k, ns):
            lhs = h0T[par].ap()[:, k * BL:(k + 1) * BL]
            def g(k=k, ns=ns, lhs=lhs):
                nc.tensor.matmul(
                    g1.ap()[64:128, ns * 512:(ns + 1) * 512],
                    lhs,
                    wih1t.ap()[:, k * G + ns * 512:k * G + (ns + 1) * 512],
                    start=False, stop=(k == 3 and ns == 3))
            return g
        k01 = [f(k, ns) for ns in range(NS) for k in (0, 1)]
        k23a = [f(k, ns) for ns in (0, 1) for k in (2, 3)]
        k23b = [f(k, ns) for ns in (2, 3) for k in (2, 3)]
        return k01, k23a, k23b

    # ---- elementwise ----

    HB = 256  # hidden half-block; gate rows are [i,f,o,g]x2 halves of 1024

    g0b = g0.bitcast(BF16)   # [128, 4096] bf16 view for transpose targets
    g1b = g1.bitcast(BF16)

    def emit_ew0_half(par, hb):
        """Layer-0 elementwise for one hidden half."""
        sg = sig0[par].ap()
        if True:
            gofs = hb * 1024
            nc.scalar.activation(sg[:, hb * 768:hb * 768 + 768],
                                 g0.ap()[0:BL, gofs:gofs + 768], AF.Sigmoid)
            nc.scalar.activation(tg0[par].ap()[:, hb * HB:hb * HB + HB],
                                 g0.ap()[0:BL, gofs + 768:gofs + 1024], AF.Tanh)
            nc.vector.tensor_mul(ta0[par].ap()[:, hb * HB:hb * HB + HB],
                                 sg[:, hb * 768:hb * 768 + HB],
                                 tg0[par].ap()[:, hb * HB:hb * HB + HB])
            nc.vector.tensor_mul(tb0[par].ap()[:, hb * HB:hb * HB + HB],
                                 sg[:, hb * 768 + HB:hb * 768 + 2 * HB],
                                 c0.ap()[:, hb * HB:hb * HB + HB])
            nc.vector.tensor_add(c0.ap()[:, hb * HB:hb * HB + HB],
                                 ta0[par].ap()[:, hb * HB:hb * HB + HB],
                                 tb0[par].ap()[:, hb * HB:hb * HB + HB])
            nc.scalar.activation(tc0[par].ap()[:, hb * HB:hb * HB + HB],
                                 c0.ap()[:, hb * HB:hb * HB + HB], AF.Tanh)
            nc.vector.tensor_mul(h0[par].ap()[:, hb * HB:hb * HB + HB],
                                 sg[:, hb * 768 + 2 * HB:hb * 768 + 3 * HB],
                                 tc0[par].ap()[:, hb * HB:hb * HB + HB])

    def emit_transpose0_half(par, hb):
        for k in (2 * hb, 2 * hb + 1):
            nc.tensor.transpose(g0b.ap()[0:128, k * BL:(k + 1) * BL],
                                h0[par].ap()[0:BL, k * 128:(k + 1) * 128],
                                identc.ap()[0:BL, :])
        nc.vector.tensor_copy(
            h0T[par].ap()[:, hb * 2 * BL:(hb + 1) * 2 * BL],
            g0b.ap()[0:128, hb * 2 * BL:(hb + 1) * 2 * BL])

    def emit_ew1_half(par, hb):
        sg = sig1[par].ap()[64:128, :]
        tg = tg1[par].ap()[64:128, :]
        cc = c1f.ap()[64:128, :]
        aa = ta1[par].ap()[64:128, :]
        bb = tb1[par].ap()[64:128, :]
        tcc = tc1[par].ap()[64:128, :]
        hh = h1[par].ap()[64:128, :]
        if True:
            gofs = hb * 1024
            nc.scalar.activation(sg[:, hb * 768:hb * 768 + 768],
                                 g1.ap()[64:128, gofs:gofs + 768], AF.Sigmoid)
            nc.scalar.activation(tg[:, hb * HB:hb * HB + HB],
                                 g1.ap()[64:128, gofs + 768:gofs + 1024],
                                 AF.Tanh)
            nc.vector.tensor_mul(aa[:, hb * HB:hb * HB + HB],
                                 sg[:, hb * 768:hb * 768 + HB],
                                 tg[:, hb * HB:hb * HB + HB])
            nc.vector.tensor_mul(bb[:, hb * HB:hb * HB + HB],
                                 sg[:, hb * 768 + HB:hb * 768 + 2 * HB],
                                 cc[:, hb * HB:hb * HB + HB])
            nc.vector.tensor_add(cc[:, hb * HB:hb * HB + HB],
                                 aa[:, hb * HB:hb * HB + HB],
                                 bb[:, hb * HB:hb * HB + HB])
            nc.scalar.activation(tcc[:, hb * HB:hb * HB + HB],
                                 cc[:, hb * HB:hb * HB + HB], AF.Tanh)
            nc.vector.tensor_mul(hh[:, hb * HB:hb * HB + HB],
                                 sg[:, hb * 768 + 2 * HB:hb * 768 + 3 * HB],
                                 tcc[:, hb * HB:hb * HB + HB])

    def emit_transpose1_half(par, hb):
        for k in (2 * hb, 2 * hb + 1):
            nc.tensor.transpose(g1b.ap()[0:128, k * BL:(k + 1) * BL],
                                h1[par].ap()[64:128, k * 128:(k + 1) * 128],
                                identc.ap()[64:128, :])
        nc.vector.tensor_copy(
            h1T[par].ap()[:, hb * 2 * BL:(hb + 1) * 2 * BL],
            g1b.ap()[0:128, hb * 2 * BL:(hb + 1) * 2 * BL])

    def emit_pred_mms(u, ks):
        par = u % 2
        pps = g1.ap()[0:1, 512:512 + BL]
        for k in ks:
            nc.tensor.matmul(pps, woutt.ap()[:, k:k + 1],
                             h1T[par].ap()[:, k * BL:(k + 1) * BL],
                             start=(k == 0), stop=(k == 3))

    def emit_pred_writes(u, pred_week, pred_dst):
        pps = g1.ap()[0:1, 512:512 + BL]
        nc.vector.tensor_scalar_add(outst.ap()[0:1, u * BL:(u + 1) * BL], pps,
                                    bout_val)
        if pred_week:
            dst_t, dst_col = pred_dst
            nc.vector.tensor_scalar_add(dst_t.ap()[0:1, dst_col:dst_col + BL],
                                        pps, bout_val)
        else:
            nc.vector.tensor_scalar_add(predl.ap(), pps, bout_val)

    def pred_dst_for(u, pred_week):
        if not pred_week:
            return None
        if (u % U) < U - 1:
            return (xbuf[(u // U) % 2], ((u % U) + 1) * BL)
        return (xbuf[(u // U + 1) % 2], 0)

    with tile.TileContext(nc) as tc:
        # ---- preamble: weights, constants, state init ----
        nc.gpsimd.memset(c0.ap(), 0.0)
        nc.gpsimd.memset(c1f.ap(), 0.0)
        nc.sync.dma_start(out=whh0t.ap(), in_=whh0t_d.ap())
        nc.sync.dma_start(out=wih1t.ap(), in_=wih1t_d.ap())
        nc.sync.dma_start(out=whh1t.ap(), in_=whh1t_d.ap())
        nc.sync.dma_start(out=wih0t.ap(), in_=wih0t_d.ap())
        nc.sync.dma_start(out=bias1.ap(), in_=bias1_d.ap())
        nc.sync.dma_start(out=woutt.ap(), in_=woutt_d.ap())
        nc.sync.dma_start(out=ones.ap(), in_=ones_d.ap())
        nc.sync.dma_start(out=identc.ap(), in_=identc_d.ap())
        for p in (0, 1):
            nc.sync.dma_start(out=h0T[p].ap(), in_=zeros_d.ap())
            nc.sync.dma_start(out=h1T[p].ap(), in_=zeros_d.ap())
            nc.sync.dma_start(out=xbuf[p].ap()[:, 0:4 * BL], in_=zeros_d.ap())
            nc.sync.dma_start(out=xbuf[p].ap()[:, 4 * BL:8 * BL],
                              in_=zeros_d.ap())

        def week_loop(week, pred_week):
            blk_base = week * WEEK // U
            it_base = week * WEEK // STEPS_PER_IT

            def body(i):
                lo = 1 if pred_week else 0

                def dma_sub(a):
                    nc.sync.dma_start(
                        out=xbuf[a % 2].ap()[lo:KX, :],
                        in_=xaug_d.ap()[ds((blk_base + SUBS_PER_IT * i + a) * KX
                                           + lo, KX - lo), :])

                dma_sub(0)
                dma_sub(1)
                run = lambda ts: [f() for f in ts]

                def step_head(u):
                    """transposes of h1(u-1), pred(u-1), staging writes —
                    runs right after EW1(u-1) completes. In original-data
                    weeks the next x-side matmuls fill the wait for the
                    second h1 half."""
                    up, parp = u - 1, (u - 1) % 2
                    emit_transpose1_half(parp, 0)
                    emit_pred_mms(up, (0, 1))
                    emit_transpose1_half(parp, 1)
                    emit_pred_mms(up, (2, 3))
                    emit_pred_writes(up, pred_week,
                                     pred_dst_for(up, pred_week))

                n_early, n_late = mm_whh0(0, True)
                run(n_early + n_late)
                _interleave(mm_xside(0, xbuf[0]), mm_bias1(0))
                for u in range(STEPS_PER_IT):
                    par = u % 2
                    if u > 0 and u % U == 0 and u // U + 1 < SUBS_PER_IT:
                        dma_sub(u // U + 1)     # prefetch the next sub-block
                    emit_ew0_half(par, 0)       # ACT/DVE
                    emit_ew0_half(par, 1)
                    run(mm_whh1(u))             # PE fills the EW0 window
                    emit_transpose0_half(par, 0)
                    w_k01, w_k23a, w_k23b = mm_wih1(u)
                    if u + 1 < STEPS_PER_IT:
                        n_early, n_late = mm_whh0(u + 1, True)
                    else:
                        n_early, n_late = [], []
                    _interleave(w_k01, n_early)
                    emit_transpose0_half(par, 1)
                    run(w_k23a)
                    emit_ew1_half(par, 0)       # ACT/DVE
                    run(w_k23b)                 # gates1-h1 producers
                    emit_ew1_half(par, 1)
                    run(n_late)                 # fills the EW1 window
                    if u + 1 < STEPS_PER_IT:
                        step_head(u + 1)
                        _interleave(mm_xside(u + 1, xbuf[((u + 1) // U) % 2]),
                                    mm_bias1(u + 1))
                step_head(STEPS_PER_IT)
                nc.sync.dma_start(out=out_d.ap()[ds(it_base + i, 1), :],
                                  in_=outst.ap())

            with tc.For_i(0, IT_PER_WEEK, 1, staggered_reset=True,
                          hint_engines=(mybir.EngineType.PE,
                                        mybir.EngineType.Activation,
                                        mybir.EngineType.DVE)) as i:
                body(i)

        week_loop(0, False)
        # pred(671) -> feat0 slot for t=672
        nc.vector.tensor_copy(xbuf[0].ap()[0:1, 0:BL], predl.ap())
        week_loop(1, True)
        week_loop(2, False)
        nc.vector.tensor_copy(xbuf[0].ap()[0:1, 0:BL], predl.ap())
        week_loop(3, True)

    nc.compile()
    return nc


def _prep_inputs(x, Wih0, Whh0, bih0, bhh0, Wih1, Whh1, bih1, bhh1, Wout, bout):
    """Host-side reshapes: gate reorder to [i,f,o,g], weight transposes,
    hi/lo bias split, per-core xaug staging layout. bf16 matmul operands."""
    f32 = np.float32
    bf16 = ml_dtypes.bfloat16
    HB = 256
    blocks = []
    for hb in range(2):
        blocks += [np.arange(0, 512)[hb*HB:(hb+1)*HB],          # i half
                   np.arange(512, 1024)[hb*HB:(hb+1)*HB],       # f half
                   np.arange(1536, 2048)[hb*HB:(hb+1)*HB],      # o half
                   np.arange(1024, 1536)[hb*HB:(hb+1)*HB]]      # g half
    perm = np.concatenate(blocks)

    def wT(w):  # [G, 512] -> [128, 4*G] chunk-k at cols [G*k, G*k+G)
        t = np.ascontiguousarray(w[perm].T.astype(f32))          # [512, G]
        return np.ascontiguousarray(
            t.reshape(4, 128, G).transpose(1, 0, 2).reshape(128, 4 * G)
        ).astype(bf16)

    def hilo(v):  # [G] fp32 -> [2, G] bf16 rows summing to ~v
        hi = v.astype(bf16).astype(f32)
        lo = (v - hi).astype(bf16)
        return np.stack([hi.astype(bf16), lo], axis=0)

    whh0t = wT(Whh0)
    wih1t = wT(Wih1)
    whh1t = wT(Whh1)
    bias0 = hilo((bih0 + bhh0)[perm].astype(f32))                # [2, G] bf16
    bias1 = np.zeros((128, G), bf16)
    bias1[0:2] = hilo((bih1 + bhh1)[perm].astype(f32))
    wih0p = Wih0[perm].astype(f32)                               # [G, 33]
    wih0t = np.zeros((128, G), bf16)
    wih0t[0:F + 1] = wih0p.T.astype(bf16)
    wih0t[F + 1:F + 3] = bias0
    woutt = np.ascontiguousarray(Wout.reshape(4, 128).T.astype(f32)).astype(bf16)

    onesp = np.zeros((128, BL), bf16)
    onesp[0:2] = 1.0

    tw = np.arange(S) // WEEK
    mask = np.where((tw % 2 == 0) & ((tw + 1) * WEEK <= S), 0.0, 1.0)
    flag = np.where((mask == 0.0) | (np.arange(S) == 0), 0.0, 1.0).astype(f32)

    xaugs = []
    for c in range(NCORES):
        xc = x[c * BL:(c + 1) * BL].astype(f32)        # [BL, S, F]
        arr = np.empty((S, KX, BL), f32)
        arr[:, 0, :] = xc[:, :, 0].T
        arr[:, 1:F, :] = xc[:, :, 1:].transpose(1, 2, 0)
        arr[:, F, :] = flag[:, None]
        arr[:, F + 1, :] = 1.0
        arr[:, F + 2, :] = 1.0
        a = arr.reshape(NBLK, U, KX, BL).transpose(0, 2, 1, 3)
        xaugs.append(np.ascontiguousarray(
            a.reshape(NBLK * KX, U * BL)).astype(bf16))

    shared = {
        "whh0t": whh0t, "wih1t": wih1t, "whh1t": whh1t, "wih0t": wih0t,
        "bias1": np.ascontiguousarray(bias1), "woutt": woutt,
        "ones": onesp, "zeros": np.zeros((128, 4 * BL), bf16),
        "identc": np.ascontiguousarray(
            np.tile(np.eye(BL, dtype=f32), (2, 1))).astype(bf16),
    }
    in_maps = [dict(shared, xaug=xaugs[c]) for c in range(NCORES)]
    return in_maps, float(np.asarray(bout).reshape(-1)[0])


def kernel(x, Wih0, Whh0, bih0, bhh0, Wih1, Whh1, bih1, bhh1, Wout, bout,
           _trace=False):
    global _LAST_RESULTS
    x = np.asarray(x)
    in_maps, bout_val = _prep_inputs(
        x, np.asarray(Wih0), np.asarray(Whh0), np.asarray(bih0),
        np.asarray(bhh0), np.asarray(Wih1), np.asarray(Whh1),
        np.asarray(bih1), np.asarray(bhh1), np.asarray(Wout),
        np.asarray(bout))
    nc = _build(bout_val, trace=_trace)
    res = run_bass_kernel_spmd(nc, in_maps, core_ids=list(range(NCORES)),
                               trace=_trace)
    _LAST_RESULTS = res
    out = np.empty((B, S, 1), np.float32)
    for c in range(NCORES):
        oc = res.results[c]["out"].reshape(S, BL)     # [S, BL]
        out[c * BL:(c + 1) * BL, :, 0] = oc.T
    return out
